# revision 1
# baseline (speedup 1.0000x reference)
"""Two-layer GAT (graph attention) kernel for 8 Trainium2 NeuronCores.

v2 strategy (destination-sharded edge parallelism, gather-prep optimized):
  * Nodes are degree-sorted and dealt round-robin to the 8 cores; each core
    aggregates messages for its own 6250 nodes only (no cross-core reduce).
  * Sharded front end: each core computes the layer-1 fat-row table for ITS
    stripe only (bf16, 512B rows: [h0|1|h1|1|h2|1|h3|1|a_src(4)|pad]), then an
    AllGather replicates the full table to every core's HBM.  The interleaved
    "ones" columns make the attention denominator fall out of the same
    slot-reduce as the messages (no separate denominator reduce).
  * Per-edge rows are fetched with dma_gather (SWDGE).  The Pool-engine
    descriptor-prep cost is linear in the static index count, so padding is
    minimized with OVERLAPPED index tables: table A = rows [0, 32768),
    table B = rows [17408, 50176) of the same tensor (int16 index range fits
    both exactly).  Edges whose source falls in the overlap are assigned to
    whichever side balances that destination's A/B slot counts.
  * A and B gathers of two consecutive destination blocks land in ONE SBUF
    tile (4 blocks worth for layer 2), halving per-gather fixed costs.
    Gathers rotate over 4 SWDGE queues (4 Q7 cpu pairs, 4 descriptor rings).
  * Padding slots point at a dummy row whose alpha is -1e30 => exp() == 0.
  * Layer 2 repeats the scheme with 256B f32 rows [h2(32)|1|a2_src|pad].

The host side (pure numpy) permutes nodes, builds the padded gather index
lists, and un-permutes the result.
"""

import sys

sys.path.insert(0, "/opt/trn_rl_repo")

import numpy as np

import concourse.bacc as bacc
import concourse.bass as bass
import concourse.mybir as mybir
import concourse.tile as tile
from concourse.bass_utils import run_bass_kernel_spmd

F32 = mybir.dt.float32
BF16 = mybir.dt.bfloat16
I16 = mybir.dt.int16
AL = mybir.AluOpType
ACT = mybir.ActivationFunctionType

CORES = 8
NEG_SLOPE = 0.2
NEG_BIG = -1.0e30

# problem constants (nn_GAT_35296041238878)
N = 50000
IN_DIM = 128
HID = 32
HEADS = 4
OUT_DIM = 32

# layer-1 fat row (bf16): [h0(32)|1|h1(32)|1|h2(32)|1|h3(32)|1|asrc(4)|pad] = 256
L1_ROW = 256
L1_USE = HEADS * (HID + 1)          # 132 (h+ones)
L1H = HEADS * HID                   # 128
W1N = L1H + 2 * HEADS               # 136 matmul cols [h|asrc|adst]
# layer-2 fat row (f32): [h2(32)|1|a2s|pad] = 64
L2_ROW = 64
L2_USE = OUT_DIM + 1                # 33
W2N = OUT_DIM + 2                   # 34 matmul cols [h2|a2s|a2d]

NQ = 4                              # SWDGE queues
L1_GRP = 2                          # dst blocks per gather, layer 1
L2_GRP = 4                          # dst blocks per gather, layer 2

_CACHE = {}

# ---------------------------------------------------------------------------
# Tile's DMASW lane round-robin is not SWDGE-queue-aware: a lane semaphore is
# locked to the queue of its first user, so rotating queue_num with the
# default assignment trips "locked to SWDGE queue" at schedule time.
# Partition the 8 lanes: queue q -> lanes [q*2, q*2+2).
import concourse.tile_sem_assignment as _tsa


def _queue_aware_assign_tick(self, inst):
    q = getattr(inst, "queue_num", None)
    if q is not None and isinstance(inst, _tsa.DMAInst) \
            and inst.engine == _tsa.mybir.EngineType.Pool:
        if not hasattr(self, "_q_lane_ctr"):
            self._q_lane_ctr = {}
        ctr = self._q_lane_ctr.get(q, 0)
        self._q_lane_ctr[q] = ctr + 1
        lanes = max(1, self.swdge_sem_count // NQ)
        self.next_sw_dma_idx = (q % NQ) * lanes + (ctr % lanes)
    return _tsa.TileClockTick._orig_assign_tick(self, inst)


if not hasattr(_tsa.TileClockTick, "_orig_assign_tick"):
    _tsa.TileClockTick._orig_assign_tick = _tsa.TileClockTick._assign_tick
    _tsa.TileClockTick._assign_tick = _queue_aware_assign_tick


# ----------------------------------------------------------------------------
# host-side graph preprocessing
# ----------------------------------------------------------------------------
def _prep_graph(edge_index, n_nodes, bpc):
    """Permute nodes, shard by destination, build padded gather index lists.

    Index tables overlap: A = rows [0, 32768), B = rows [BBASE, tbl_rows)
    with BBASE = tbl_rows - 32768.  Edges with src pos in the overlap are
    assigned to balance each node's A/B slot counts.
    """
    npc = n_nodes // CORES           # real nodes per core
    stride = bpc * 128               # table stripe per core (rows >= npc: dummy)
    tbl_rows = CORES * stride
    bbase = tbl_rows - 32768
    assert npc < stride and bbase >= 0 and tbl_rows - bbase == 32768
    a_dummy = npc                    # core-0 stripe dummy row, < 32768
    bd_core = next(c for c in range(CORES) if c * stride + npc >= bbase)
    b_dummy_local = bd_core * stride + npc - bbase
    assert 0 <= b_dummy_local < 32768

    src = np.concatenate([edge_index[0], np.arange(n_nodes)]).astype(np.int64)
    dst = np.concatenate([edge_index[1], np.arange(n_nodes)]).astype(np.int64)

    deg = np.bincount(dst, minlength=n_nodes)
    order = np.argsort(-deg, kind="stable")
    # rank r -> core r%8, local row r//8  (degree-balanced, within-core sorted)
    pos = np.empty(n_nodes, dtype=np.int64)
    ranks = np.arange(n_nodes)
    pos[order] = (ranks % CORES) * stride + ranks // CORES
    nodes_of_core = [order[c::CORES] for c in range(CORES)]

    dpos = pos[dst]
    e_core = dpos // stride
    ld = dpos % stride               # local dst row, < npc
    sp = pos[src]                    # source table position

    # ---- balanced A/B side assignment ----
    key = e_core * stride + ld       # destination node's table row
    fixedB = sp >= 32768
    flex = (sp >= bbase) & ~fixedB
    degn = np.bincount(key, minlength=tbl_rows)
    nA_fixed = np.bincount(key[sp < bbase], minlength=tbl_rows)
    nF = np.bincount(key[flex], minlength=tbl_rows)
    tgtA = np.minimum(np.maximum((degn + 1) // 2, nA_fixed), nA_fixed + nF)
    # rank of each flex edge within its key
    fidx = np.flatnonzero(flex)
    o = np.argsort(key[fidx], kind="stable")
    fs = fidx[o]
    ks = key[fs]
    change = np.r_[True, ks[1:] != ks[:-1]]
    starts = np.flatnonzero(change)
    gid = np.cumsum(change) - 1
    frank = np.arange(len(fs)) - starts[gid]
    sideB = fixedB.copy()
    sideB[fs] = frank >= (tgtA - nA_fixed)[ks]

    nA = np.bincount(key[~sideB], minlength=tbl_rows)
    nB = degn - nA

    def blockmax(x):
        return x.reshape(CORES, bpc, 128).max(axis=0).max(axis=1)

    da = np.maximum(blockmax(nA), 1)
    db = np.maximum(blockmax(nB), 1)
    offa = np.concatenate([[0], np.cumsum(da)])
    offb = np.concatenate([[0], np.cumsum(db)])

    idxa_list, idxb_list = [], []
    for c in range(CORES):
        m = e_core == c
        ldc, spc, sbc = ld[m], sp[m], sideB[m]
        o2 = np.lexsort((sbc, ldc))
        ldc, spc, sbc = ldc[o2], spc[o2], sbc[o2]
        keyc = ldc * 2 + sbc
        change = np.r_[True, keyc[1:] != keyc[:-1]]
        gid = np.cumsum(change) - 1
        starts = np.flatnonzero(change)
        jj = np.arange(len(ldc)) - starts[gid]
        bidx = ldc // 128
        d = ldc % 128
        flat_a = np.full(128 * offa[-1], a_dummy, dtype=np.int64)
        flat_b = np.full(128 * offb[-1], b_dummy_local, dtype=np.int64)
        ma = ~sbc
        flat_a[(offa[bidx[ma]] + jj[ma]) * 128 + d[ma]] = spc[ma]
        mb = sbc
        flat_b[(offb[bidx[mb]] + jj[mb]) * 128 + d[mb]] = spc[mb] - bbase
        assert flat_a.max() < 32768 and flat_b.max() < 32768
        # wrap per block: i -> [i%16, i//16], concat blocks along columns
        wa = np.concatenate(
            [flat_a[128 * offa[b]:128 * offa[b + 1]].reshape(-1, 16).T
             for b in range(bpc)], axis=1).astype(np.int16)
        wb = np.concatenate(
            [flat_b[128 * offb[b]:128 * offb[b + 1]].reshape(-1, 16).T
             for b in range(bpc)], axis=1).astype(np.int16)
        idxa_list.append(np.tile(wa, (8, 1)))
        idxb_list.append(np.tile(wb, (8, 1)))

    return dict(
        npc=npc, stride=stride, tbl_rows=tbl_rows, bbase=bbase, bpc=bpc,
        da=da.astype(int).tolist(), db=db.astype(int).tolist(),
        offa=offa.astype(int).tolist(), offb=offb.astype(int).tolist(),
        pos=pos, nodes_of_core=nodes_of_core,
        idxa=idxa_list, idxb=idxb_list,
    )


# ----------------------------------------------------------------------------
# device program
# ----------------------------------------------------------------------------
def _build_program(g, has_b1):
    bpc, stride, tbl_rows, bbase = g["bpc"], g["stride"], g["tbl_rows"], g["bbase"]
    da, db, offa, offb = g["da"], g["db"], g["offa"], g["offb"]
    npc = g["npc"]
    sa_cols = 8 * offa[-1]
    sb_cols = 8 * offb[-1]

    nc = bacc.Bacc("TRN2", target_bir_lowering=False, debug=False,
                   num_devices=CORES, num_swdge_queues=NQ)

    xTs = nc.dram_tensor("xTs", [128, stride], BF16, kind="ExternalInput")
    w1e = nc.dram_tensor("w1e", [128, W1N], BF16, kind="ExternalInput")
    w2e = nc.dram_tensor("w2e", [L1H, W2N], BF16, kind="ExternalInput")
    b1t = nc.dram_tensor("b1t", [128, L1H], F32, kind="ExternalInput")
    ident = nc.dram_tensor("ident", [128, 128], F32, kind="ExternalInput")
    idxa = nc.dram_tensor("idxa", [128, sa_cols], I16, kind="ExternalInput")
    idxb = nc.dram_tensor("idxb", [128, sb_cols], I16, kind="ExternalInput")

    cc1 = nc.dram_tensor("cc1", [stride, L1_ROW], BF16)
    tbl1 = nc.dram_tensor("tbl1", [tbl_rows, L1_ROW], BF16, addr_space="Shared")
    cc2 = nc.dram_tensor("cc2", [stride, L2_ROW], F32)
    tbl2 = nc.dram_tensor("tbl2", [tbl_rows, L2_ROW], F32, addr_space="Shared")
    out = nc.dram_tensor("out", [stride, OUT_DIM], F32, kind="ExternalOutput")

    with tile.TileContext(nc) as tc:
        with (
            tc.tile_pool(name="res", bufs=1) as res,
            tc.tile_pool(name="ps", bufs=2, space="PSUM") as psp,
            tc.tile_pool(name="sml", bufs=2) as sml,
        ):
            # ---- resident constants ----
            w1e_t = res.tile([128, W1N], BF16, tag="w1e")
            nc.sync.dma_start(w1e_t[:], w1e.ap())
            w2e_t = res.tile([L1H, W2N], BF16, tag="w2e")
            nc.sync.dma_start(w2e_t[:], w2e.ap())
            b1_t = res.tile([128, L1H], F32, tag="b1")
            nc.sync.dma_start(b1_t[:], b1t.ap())
            id_t = res.tile([128, 128], F32, tag="ident")
            nc.sync.dma_start(id_t[:], ident.ap())
            ia_t = res.tile([128, sa_cols], I16, tag="idxa")
            nc.sync.dma_start(ia_t[:], idxa.ap())
            ib_t = res.tile([128, sb_cols], I16, tag="idxb")
            nc.sync.dma_start(ib_t[:], idxb.ap())
            ad_own = res.tile([128, bpc * HEADS], F32, tag="adown")
            ad2_own = res.tile([128, bpc], F32, tag="ad2own")

            # dummy rows [npc, stride) of both cc tensors: alpha = -1e30
            pad_rows = stride - npc
            dmy1 = res.tile([pad_rows, L1_ROW], BF16, tag="dmy1")
            nc.vector.memset(dmy1[:], 0.0)
            nc.vector.memset(dmy1[:, L1_USE:L1_USE + HEADS], NEG_BIG)
            nc.sync.dma_start(cc1.ap()[npc:stride, :], dmy1[:])
            # l2 row layout: [h2(0:32) | a2s(32) | one(33) | pad]
            dmy2 = res.tile([pad_rows, L2_ROW], F32, tag="dmy2")
            nc.vector.memset(dmy2[:], 0.0)
            nc.vector.memset(dmy2[:, OUT_DIM:OUT_DIM + 1], NEG_BIG)
            nc.sync.dma_start(cc2.ap()[npc:stride, :], dmy2[:])

            # ---- front end: this core's stripe of the fat-row table ----
            fe_ctx = tc.tile_pool(name="fe", bufs=3)
            fe = fe_ctx.__enter__()
            FCH = 4                   # blocks per cc1 write
            # pre-zero the 3 rotating fat buffers once; pads stay zero
            for _ in range(3):
                f0 = fe.tile([128, FCH, L1_ROW], BF16, tag="fat")
                nc.vector.memset(f0[:].rearrange("p a b -> p (a b)"), 0.0)
            tbl1_v = tbl1.ap().rearrange("(c s) e -> c s e", c=CORES)
            CH_ROWS = 1536            # AG chunk rows (12 frontend blocks)
            ag1_fired = 0

            for t0 in range(0, bpc, FCH):
                tn = min(FCH, bpc - t0)
                fat = fe.tile([128, FCH, L1_ROW], BF16, tag="fat")
                for k in range(tn):
                    t = t0 + k
                    xt = fe.tile([128, 128], BF16, tag="xt")
                    nc.sync.dma_start(xt[:], xTs.ap()[:, 128 * t:128 * (t + 1)])
                    ps = psp.tile([128, W1N], F32, tag="feps")
                    nc.tensor.matmul(ps[:], xt[:], w1e_t[:], start=True, stop=True)
                    fk = fat[:, k, :]
                    f4 = fk[:, 0:L1_USE].rearrange("p (h c) -> p h c", h=HEADS)
                    nc.vector.tensor_copy(
                        f4[:, :, 0:HID],
                        ps[:, 0:L1H].rearrange("p (h c) -> p h c", h=HEADS))
                    nc.vector.memset(f4[:, :, HID:HID + 1], 1.0)
                    nc.vector.tensor_copy(
                        fk[:, L1_USE:L1_USE + HEADS], ps[:, L1H:L1H + HEADS])
                    nc.vector.tensor_copy(
                        ad_own[:, HEADS * t:HEADS * (t + 1)],
                        ps[:, L1H + HEADS:L1H + 2 * HEADS])
                nrows = min(128 * tn, npc - 128 * t0)
                if nrows == 128 * tn:
                    nc.sync.dma_start(
                        cc1.ap()[128 * t0:128 * t0 + nrows, :].rearrange(
                            "(t p) e -> p t e", p=128), fat[:, 0:tn, :])
                else:
                    for k in range(tn):
                        t = t0 + k
                        nr = min(128, npc - 128 * t)
                        if nr > 0:
                            nc.sync.dma_start(
                                cc1.ap()[128 * t:128 * t + nr, :],
                                fat[0:nr, k, :])

            fe_ctx.__exit__(None, None, None)
            tc.strict_bb_all_engine_barrier()
            nc.gpsimd.collective_compute(
                "AllGather", AL.bypass,
                replica_groups=[list(range(CORES))],
                ins=[cc1.ap().opt()], outs=[tbl1.ap().opt()])
            tc.strict_bb_all_engine_barrier()

            # ---- layer 1: per-block gathers, tree slot-reduce ----
            l1_gat_ctx = tc.tile_pool(name="gat1", bufs=3)
            gat = l1_gat_ctx.__enter__()
            l1_mid_ctx = tc.tile_pool(name="mid1", bufs=3)
            mid = l1_mid_ctx.__enter__()
            # persistent double-buffered l2fat with constant cols pre-set
            l2f_tiles = []
            for i in range(2):
                lf = res.tile([128, L2_ROW], F32, tag=f"l2f{i}")
                nc.vector.memset(lf[:, OUT_DIM + 2:L2_ROW], 0.0)
                nc.vector.memset(lf[:, OUT_DIM + 1:OUT_DIM + 2], 1.0)
                l2f_tiles.append(lf)
            tblA = tbl1.ap()[0:32768, :]
            tblB = tbl1.ap()[bbase:tbl_rows, :]
            tbl2_v = tbl2.ap().rearrange("(c s) e -> c s e", c=CORES)
            ag2_fired = 0

            def tree_reduce(m, D, W):
                """In-place pairwise slot reduce of m[:, 0:D, 0:W] -> m[:,0,:].

                All adds are on flat contiguous [128, k*W] slabs.
                """
                Dt = 1 << (D.bit_length() - 1)
                if Dt == D and D > 1:
                    Dt >>= 1
                if D > Dt:
                    k = D - Dt
                    nc.vector.tensor_tensor(
                        m[:, 0:k, :].rearrange("p a b -> p (a b)"),
                        m[:, 0:k, :].rearrange("p a b -> p (a b)"),
                        m[:, Dt:D, :].rearrange("p a b -> p (a b)"), AL.add)
                k = Dt >> 1
                while k >= 1:
                    nc.vector.tensor_tensor(
                        m[:, 0:k, :].rearrange("p a b -> p (a b)"),
                        m[:, 0:k, :].rearrange("p a b -> p (a b)"),
                        m[:, k:2 * k, :].rearrange("p a b -> p (a b)"), AL.add)
                    k >>= 1

            def split_gathers(out_tile, tblA_ap, tblB_ap, idx_a, idx_b,
                              b, row, q0):
                """4 gathers per block (A and B halves) on 4 distinct queues."""
                DA, DB = da[b], db[b]
                parts = []
                hA = DA // 2
                if hA >= 1:
                    parts.append((out_tile[:, 0:hA, :], tblA_ap,
                                  idx_a[:, 8 * offa[b]:8 * (offa[b] + hA)], hA))
                    parts.append((out_tile[:, hA:DA, :], tblA_ap,
                                  idx_a[:, 8 * (offa[b] + hA):8 * offa[b + 1]],
                                  DA - hA))
                else:
                    parts.append((out_tile[:, 0:DA, :], tblA_ap,
                                  idx_a[:, 8 * offa[b]:8 * offa[b + 1]], DA))
                hB = DB // 2
                if hB >= 1:
                    parts.append((out_tile[:, DA:DA + hB, :], tblB_ap,
                                  idx_b[:, 8 * offb[b]:8 * (offb[b] + hB)], hB))
                    parts.append((out_tile[:, DA + hB:DA + DB, :], tblB_ap,
                                  idx_b[:, 8 * (offb[b] + hB):8 * offb[b + 1]],
                                  DB - hB))
                else:
                    parts.append((out_tile[:, DA:DA + DB, :], tblB_ap,
                                  idx_b[:, 8 * offb[b]:8 * offb[b + 1]], DB))
                for i, (oap, tap, iap, dn) in enumerate(parts):
                    nc.gpsimd.dma_gather(
                        oap, tap, iap, 128 * dn, 128 * dn, row,
                        elem_step=row, single_packet=False,
                        queue_num=(q0 + i) % NQ)

            for b in range(bpc):
                DA, DB = da[b], db[b]
                D = DA + DB
                gt = gat.tile([128, D, L1_ROW], BF16, tag="g")
                split_gathers(gt, tblA, tblB, ia_t, ib_t, b, L1_ROW, b % NQ)

                adb = ad_own[:, HEADS * b:HEADS * (b + 1)]
                z = sml.tile([128, D, HEADS], F32, tag="z")
                nc.vector.tensor_tensor(
                    z[:, :, :], gt[:, :, L1_USE:L1_USE + HEADS],
                    adb.unsqueeze(1).broadcast_to([128, D, HEADS]), AL.add)
                z2 = sml.tile([128, D, HEADS], F32, tag="z2")
                nc.vector.scalar_tensor_tensor(
                    z2[:].rearrange("p a b -> p (a b)"),
                    z[:].rearrange("p a b -> p (a b)"), NEG_SLOPE,
                    z[:].rearrange("p a b -> p (a b)"),
                    op0=AL.mult, op1=AL.max)
                wb = sml.tile([128, D, HEADS], BF16, tag="wb")
                nc.scalar.activation(
                    wb[:].rearrange("p a b -> p (a b)"),
                    z2[:].rearrange("p a b -> p (a b)"), ACT.Exp)

                m = mid.tile([128, D, L1_USE], F32, tag="m")
                m4 = m[:, :, :].rearrange("p d (h c) -> p d h c", h=HEADS)
                nc.vector.tensor_tensor(
                    m4, gt[:, :, 0:L1_USE].rearrange(
                        "p d (h c) -> p d h c", h=HEADS),
                    wb[:, :, :].unsqueeze(3).broadcast_to(
                        [128, D, HEADS, HID + 1]), AL.mult)
                tree_reduce(m, D, L1_USE)
                r4 = m[:, 0, :].rearrange("p (h c) -> p h c", h=HEADS)

                rec = sml.tile([128, HEADS], F32, tag="rec")
                nc.vector.reciprocal(rec[:], r4[:, :, HID])
                o1 = sml.tile([128, L1H], F32, tag="o1")
                nc.vector.tensor_tensor(
                    o1[:].rearrange("p (h c) -> p h c", h=HEADS),
                    r4[:, :, 0:HID],
                    rec[:].unsqueeze(2).broadcast_to([128, HEADS, HID]),
                    AL.mult)
                if has_b1:
                    o1b = sml.tile([128, L1H], F32, tag="o1b")
                    nc.vector.tensor_tensor(o1b[:], o1[:], b1_t[:, :], AL.add)
                else:
                    o1b = o1
                # elu(x) = max(x, exp(min(x,0)) - 1);  e1n = -min(x,0) = relu(-x)
                e1n = sml.tile([128, L1H], F32, tag="e1n")
                nc.scalar.activation(e1n[:], o1b[:], ACT.Relu, scale=-1.0)
                e2 = sml.tile([128, L1H], F32, tag="e2")
                nc.scalar.activation(e2[:], e1n[:], ACT.Exp, scale=-1.0)
                elu = sml.tile([128, L1H], F32, tag="elu")
                nc.vector.scalar_tensor_tensor(
                    elu[:], e2[:], -1.0, o1b[:], op0=AL.add, op1=AL.max)
                # h2' = elu^T @ W2ext
                tp = psp.tile([128, 128], F32, tag="tp")
                nc.tensor.transpose(tp[:], elu[:], id_t[:])
                eluT = sml.tile([128, 128], BF16, tag="eluT")
                nc.scalar.activation(eluT[:], tp[:], ACT.Copy)
                h2p = psp.tile([128, W2N], F32, tag="h2p")
                nc.tensor.matmul(h2p[:], eluT[:], w2e_t[:],
                                 start=True, stop=True)
                l2fat = l2f_tiles[b % 2]
                # l2fat row: [h2(0:32) | a2s(32) | one(33) | pad]
                nc.scalar.activation(
                    l2fat[:, 0:OUT_DIM + 1], h2p[:, 0:OUT_DIM + 1], ACT.Copy)
                nc.scalar.activation(
                    ad2_own[:, b:b + 1], h2p[:, OUT_DIM + 1:OUT_DIM + 2],
                    ACT.Copy)
                nrows = min(128, npc - 128 * b)
                nc.sync.dma_start(
                    cc2.ap()[128 * b:128 * b + nrows, :], l2fat[0:nrows, :])

            l1_mid_ctx.__exit__(None, None, None)
            l1_gat_ctx.__exit__(None, None, None)
            tc.strict_bb_all_engine_barrier()
            nc.gpsimd.collective_compute(
                "AllGather", AL.bypass,
                replica_groups=[list(range(CORES))],
                ins=[cc2.ap().opt()], outs=[tbl2.ap().opt()])
            tc.strict_bb_all_engine_barrier()

            # ---- layer 2: per-block gathers, tree slot-reduce ----
            l2_gat_ctx = tc.tile_pool(name="gat2", bufs=4)
            gat = l2_gat_ctx.__enter__()
            l2_mid_ctx = tc.tile_pool(name="mid2", bufs=3)
            mid = l2_mid_ctx.__enter__()
            t2A = tbl2.ap()[0:32768, :]
            t2B = tbl2.ap()[bbase:tbl_rows, :]
            W2R = OUT_DIM + 2      # reduce width: [h2|a2s(junk)|one]
            for b in range(bpc):
                DA, DB = da[b], db[b]
                D = DA + DB
                g2 = gat.tile([128, D, L2_ROW], F32, tag="g2")
                split_gathers(g2, t2A, t2B, ia_t, ib_t, b, L2_ROW, b % NQ)

                # z = a2s[src] + a2d[dst] on the scalar engine (strided read)
                z = sml.tile([128, D], F32, tag="z2l")
                nc.scalar.activation(
                    z[:, :], g2[:, :, OUT_DIM], ACT.Identity,
                    bias=ad2_own[:, b:b + 1])
                z2 = sml.tile([128, D], F32, tag="z2l2")
                nc.vector.scalar_tensor_tensor(
                    z2[:, :], z[:, :], NEG_SLOPE, z[:, :],
                    op0=AL.mult, op1=AL.max)
                w2t = sml.tile([128, D], F32, tag="w2t")
                nc.scalar.activation(w2t[:, :], z2[:, :], ACT.Exp)

                m2 = mid.tile([128, D, W2R], F32, tag="m2")
                nc.vector.tensor_tensor(
                    m2[:, :, :], g2[:, :, 0:W2R],
                    w2t[:, :].unsqueeze(2).broadcast_to([128, D, W2R]),
                    AL.mult)
                tree_reduce(m2, D, W2R)
                r = m2[:, 0, :]

                rec = sml.tile([128, 1], F32, tag="rec2")
                nc.vector.reciprocal(rec[:], r[:, OUT_DIM + 1:OUT_DIM + 2])
                o2 = sml.tile([128, OUT_DIM], F32, tag="o2")
                nc.vector.tensor_scalar(
                    o2[:], r[:, 0:OUT_DIM], rec[:], None, op0=AL.mult)
                nrows = min(128, npc - 128 * b)
                nc.sync.dma_start(
                    out.ap()[128 * b:128 * b + nrows, :], o2[0:nrows, :])

            l2_mid_ctx.__exit__(None, None, None)
            l2_gat_ctx.__exit__(None, None, None)

    nc.compile()
    return nc


# ----------------------------------------------------------------------------
# weight prep + end-to-end run
# ----------------------------------------------------------------------------
def _run(x, edge_index, W1, a1_src, a1_dst, b1, W2, a2_src, a2_dst, b2,
         n_nodes, bpc, trace=False):
    x = np.asarray(x, dtype=np.float32)
    edge_index = np.asarray(edge_index)

    g = _prep_graph(edge_index, n_nodes, bpc)

    has_b1 = bool(np.abs(np.asarray(b1)).max() > 0)
    key = (4, n_nodes, bpc, has_b1, tuple(g["da"]), tuple(g["db"]))
    if key in _CACHE:
        nc = _CACHE[key]
    else:
        nc = _build_program(g, has_b1)
        _CACHE[key] = nc

    heads, hid = HEADS, HID
    W1 = np.asarray(W1, np.float32)
    W2 = np.asarray(W2, np.float32)
    w1s = np.stack([W1[:, h * hid:(h + 1) * hid] @ np.asarray(a1_src, np.float32)[h]
                    for h in range(heads)], axis=1)
    w1d = np.stack([W1[:, h * hid:(h + 1) * hid] @ np.asarray(a1_dst, np.float32)[h]
                    for h in range(heads)], axis=1)
    w1e_np = np.concatenate([W1, w1s, w1d], axis=1)
    w2s = (W2 @ np.asarray(a2_src, np.float32)[0])[:, None]
    w2d = (W2 @ np.asarray(a2_dst, np.float32)[0])[:, None]
    w2e_np = np.concatenate([W2, w2s, w2d], axis=1)

    # permuted xT (full), zero-padded; per-core stripes sliced below
    tbl_rows = g["tbl_rows"]
    stride = g["stride"]
    xT = np.zeros((IN_DIM, tbl_rows), dtype=np.float32)
    xT[:, g["pos"]] = x.T

    common = {
        "w1e": _bf16(w1e_np),
        "w2e": _bf16(w2e_np),
        "b1t": np.tile(np.asarray(b1, np.float32)[None, :], (128, 1)),
        "ident": np.eye(128, dtype=np.float32),
    }
    in_maps = []
    for c in range(CORES):
        in_maps.append({
            **common,
            "xTs": _bf16(xT[:, c * stride:(c + 1) * stride]),
            "idxa": g["idxa"][c], "idxb": g["idxb"][c],
        })

    res = run_bass_kernel_spmd(nc, in_maps, list(range(CORES)), trace=trace)

    out_full = np.empty((n_nodes, OUT_DIM), dtype=np.float32)
    npc = g["npc"]
    for c in range(CORES):
        out_full[g["nodes_of_core"][c]] = res.results[c]["out"][0:npc]
    out_full += np.asarray(b2, np.float32)[None, :]
    return out_full, res


def _bf16(a):
    import ml_dtypes
    return np.asarray(a, dtype=np.float32).astype(ml_dtypes.bfloat16)


def kernel(x, edge_index, W1, a1_src, a1_dst, b1, W2, a2_src, a2_dst, b2):
    out, _ = _run(x, edge_index, W1, a1_src, a1_dst, b1, W2, a2_src, a2_dst,
                  b2, n_nodes=N, bpc=49)
    return out



# revision 4
# speedup vs baseline: 1.2874x; 1.2874x over previous
"""Two-layer GAT kernel for 8 Trainium2 NeuronCores — v3.

v3 strategy (4-bin gathers, grouped blocks, in-place fp16 slot reduce):
  * Nodes degree-sorted, dealt round-robin to 8 cores; each core aggregates
    messages for its 6250 nodes (dst-sharded, no cross-core reduce).
  * Sharded front end computes each core's stripe of a fat-row table
    (fp16, 512B rows: [h(128) | asrc(4) | ones(4) | junk]); AllGather
    replicates it.  Layer 2 repeats with 256B fp16 rows
    [h2(32) | a2s | one | junk].
  * Per-edge rows fetched with dma_gather (SWDGE, int16 indices < 32768).
    FOUR index bins per block group make every edge 2-3 way assignable,
    minimizing the per-block slot maxima:
      A: rows [0, 32768)        stride 512B
      B: rows [17408, 50176)    stride 512B
      E: even rows (any)        stride 1024B (idx = row>>1)
      O: odd rows (any)         stride 1024B from +512B base
    Bin capacities per group are chosen by an exact small LP (Hall
    feasibility over the 16 bin subsets); blocks are grouped by a DP that
    trades gather fixed cost against slot padding.
  * Gathers are grouped (one gather per bin per group of 3-6 blocks) to
    amortize the ~1.2us SWDGE fixed cost, rotating 4 SWDGE queues.
  * Attention weights are multiplied into the gathered rows IN PLACE
    (fp16, 2x DVE rate), then a pairwise tree reduce folds slots per
    block; static "ones" columns produce the softmax denominator in the
    same reduce.  Padding slots point at dummy rows with asrc = -30000
    (exp == 0 in fp16).
"""

import sys

sys.path.insert(0, "/opt/trn_rl_repo")

import numpy as np

import concourse.bacc as bacc
import concourse.bass as bass
import concourse.mybir as mybir
import concourse.tile as tile
from concourse.bass_utils import run_bass_kernel_spmd

F32 = mybir.dt.float32
F16 = mybir.dt.float16
I16 = mybir.dt.int16
AL = mybir.AluOpType
ACT = mybir.ActivationFunctionType

CORES = 8
NEG_SLOPE = 0.2
NEG_BIG = -30000.0          # fp16-safe; exp(leaky(-30000)) == 0

# problem constants (nn_GAT_35296041238878)
N = 50000
IN_DIM = 128
HID = 32
HEADS = 4
OUT_DIM = 32

BPC = 49
STRIDE = BPC * 128           # 6272 table rows per core
TBL_ROWS = CORES * STRIDE    # 50176
BBASE = TBL_ROWS - 32768     # 17408, window-B base
NPC = N // CORES             # 6250 real nodes per core

# layer-1 row (fp16, 512B): [h(0:128) | asrc(128:132) | ones(132:136) | junk]
L1_ROW = 256
L1H = HEADS * HID            # 128
W1N = L1H + 2 * HEADS        # 136 matmul cols [h | asrc | adst]
# layer-2 row (fp16, 256B): [h2(0:32) | a2s(32) | one(33) | junk]
L2_ROW = 128
W2N = OUT_DIM + 2            # 34 matmul cols [h2 | a2s | a2d]

NQ = 4                       # SWDGE queues
SMAX = 100                   # max slots per group tile (SBUF bound)
GROUP_FIX = 12               # DP: group fixed cost in slot units

_CACHE = {}

# ---------------------------------------------------------------------------
# Tile's DMASW lane round-robin is not SWDGE-queue-aware: partition the 8
# lanes so queue q uses lanes [q*2, q*2+2).
import concourse.tile_sem_assignment as _tsa


def _queue_aware_assign_tick(self, inst):
    q = getattr(inst, "queue_num", None)
    if q is not None and isinstance(inst, _tsa.DMAInst) \
            and inst.engine == _tsa.mybir.EngineType.Pool:
        if not hasattr(self, "_q_lane_ctr"):
            self._q_lane_ctr = {}
        ctr = self._q_lane_ctr.get(q, 0)
        self._q_lane_ctr[q] = ctr + 1
        lanes = max(1, self.swdge_sem_count // NQ)
        self.next_sw_dma_idx = (q % NQ) * lanes + (ctr % lanes)
    return _tsa.TileClockTick._orig_assign_tick(self, inst)


if not hasattr(_tsa.TileClockTick, "_orig_assign_tick"):
    _tsa.TileClockTick._orig_assign_tick = _tsa.TileClockTick._assign_tick
    _tsa.TileClockTick._assign_tick = _queue_aware_assign_tick


# ---------------------------------------------------------------------------
# host-side graph preprocessing
# ---------------------------------------------------------------------------
_COPT = np.array([1 | 4, 1 | 8, 2 | 4, 2 | 8, 1 | 2 | 4, 1 | 2 | 8])
_SUBSEL = [[(int(_COPT[c]) & ~S) == 0 for c in range(6)] for S in range(16)]


def _opt_caps(M):
    """Min DA+DB+DE+DO s.t. sum over S-subset >= M[S] for all 16 subsets."""
    for T in range(int(M[15]), int(M[15]) + 16):
        for DA in range(int(M[1]), T + 1):
            for DB in range(int(M[2]), T - DA + 1):
                for DE in range(int(M[4]), T - DA - DB + 1):
                    DO = T - DA - DB - DE
                    if DO < M[8]:
                        continue
                    D = (DA, DB, DE, DO)
                    ok = True
                    for S in range(16):
                        cap = 0
                        for i in range(4):
                            if S >> i & 1:
                                cap += D[i]
                        if cap < M[S]:
                            ok = False
                            break
                    if ok:
                        return T, D
    raise RuntimeError("cap search failed")


def _assign_bins(C, caps):
    """Per-node edge->bin counts. C: [n,6] class counts (AE,AO,BE,BO,FE,FO).
    Returns [n,6] arrays: for each class, how many go to the PARITY bin
    (classes 0-3) / how many flex go to A (classes 4,5 -> (toE/toO, toA)).
    Output: eAE,oAO,eBE,oBO (fixed classes sent to parity), fEe,fOo
    (flex sent to parity), fA (flex sent to A; rest of flex goes to B)."""
    DA, DB, DE, DO = caps
    cAE, cAO, cBE, cBO, cFE, cFO = [C[:, i].astype(np.int64) for i in range(6)]
    needA = np.maximum(0, cAE + cAO - DA)
    needB = np.maximum(0, cBE + cBO - DB)
    eAE = np.minimum(cAE, needA)
    oAO = needA - eAE
    eBE = np.minimum(cBE, needB)
    oBO = needB - eBE
    # parity cap overflow: shift A-relief / B-relief between E and O
    for _ in range(2):
        xsE = np.maximum(0, (eAE + eBE) - DE)
        # shift AE->AO
        s1 = np.minimum(np.minimum(eAE, cAO - oAO), xsE)
        eAE -= s1
        oAO += s1
        xsE -= s1
        # shift BE->BO
        s2 = np.minimum(np.minimum(eBE, cBO - oBO), xsE)
        eBE -= s2
        oBO += s2
        xsO = np.maximum(0, (oAO + oBO) - DO)
        s3 = np.minimum(np.minimum(oAO, cAE - eAE), xsO)
        oAO -= s3
        eAE += s3
        xsO -= s3
        s4 = np.minimum(np.minimum(oBO, cBE - eBE), xsO)
        oBO -= s4
        eBE += s4
    # flex placement: parity first, then A, then B
    slackE = DE - (eAE + eBE)
    slackO = DO - (oAO + oBO)
    slackA = DA - (cAE - eAE + cAO - oAO)
    slackB = DB - (cBE - eBE + cBO - oBO)
    fEe = np.minimum(cFE, np.maximum(0, slackE))
    fOo = np.minimum(cFO, np.maximum(0, slackO))
    rest = (cFE - fEe) + (cFO - fOo)
    fA = np.minimum(rest, np.maximum(0, slackA))
    # remainder goes to B implicitly
    loads = np.stack([
        cAE - eAE + cAO - oAO + fA,
        cBE - eBE + cBO - oBO + (rest - fA),
        eAE + eBE + fEe,
        oAO + oBO + fOo,
    ], axis=1)
    return (eAE, oAO, eBE, oBO, fEe, fOo, fA), loads


def _assign_node_flow(counts, caps):
    """Exact per-node assignment by tiny augmenting flow. counts: [6]."""
    DA, DB, DE, DO = caps
    cap = [DA, DB, DE, DO]
    load = [0, 0, 0, 0]
    opts = [(0, 2), (0, 3), (1, 2), (1, 3), (0, 1, 2), (0, 1, 3)]
    assign = [[0] * 4 for _ in range(6)]
    for c in range(6):
        for _ in range(int(counts[c])):
            placed = False
            for b in opts[c]:
                if load[b] < cap[b]:
                    load[b] += 1
                    assign[c][b] += 1
                    placed = True
                    break
            if placed:
                continue
            # augment: move one edge of some class c2 from bin b (an option
            # of c) to another bin b2 of c2 with room
            done = False
            for b in opts[c]:
                for c2 in range(6):
                    if done or assign[c2][b] == 0:
                        continue
                    for b2 in opts[c2]:
                        if b2 != b and load[b2] < cap[b2]:
                            assign[c2][b] -= 1
                            assign[c2][b2] += 1
                            load[b2] += 1
                            assign[c][b] += 1
                            done = True
                            break
                if done:
                    break
            if not done:
                raise RuntimeError("node assignment infeasible")
    return assign


def _prep_graph(edge_index, n_nodes):
    src = np.concatenate([edge_index[0], np.arange(n_nodes)]).astype(np.int64)
    dst = np.concatenate([edge_index[1], np.arange(n_nodes)]).astype(np.int64)

    deg = np.bincount(dst, minlength=n_nodes)
    order = np.argsort(-deg, kind="stable")
    pos = np.empty(n_nodes, dtype=np.int64)
    ranks = np.arange(n_nodes)
    pos[order] = (ranks % CORES) * STRIDE + ranks // CORES
    nodes_of_core = [order[c::CORES] for c in range(CORES)]

    dpos = pos[dst]
    sp = pos[src]
    blk = (dpos % STRIDE) // 128

    Acap = sp < 32768
    Bcap = sp >= BBASE
    par = (sp & 1).astype(np.int64)
    flex = Acap & Bcap
    onlyA = Acap & ~Bcap
    cls = np.where(flex, 4 + par, np.where(onlyA, 0 + par, 2 + par))

    CC = np.zeros((TBL_ROWS, 6), dtype=np.int32)
    np.add.at(CC, (dpos, cls), 1)

    nodeblk = (np.arange(TBL_ROWS) % STRIDE) // 128
    Mb = np.zeros((BPC, 16), dtype=np.int64)
    for S in range(1, 16):
        sel = _SUBSEL[S]
        if any(sel):
            dem = CC[:, sel].sum(axis=1)
            for b in range(BPC):
                Mb[b, S] = dem[nodeblk == b].max()

    # caps for every candidate contiguous block group of size <= 8
    Tij = {}
    for i in range(BPC):
        M = np.zeros(16, dtype=np.int64)
        for j in range(i, min(i + 8, BPC)):
            M = np.maximum(M, Mb[j])
            Tij[(i, j)] = _opt_caps(M)

    # DP over group boundaries
    INF = 1 << 30
    dp = [INF] * (BPC + 1)
    dp[0] = 0
    parent = [0] * (BPC + 1)
    for j in range(1, BPC + 1):
        for i in range(max(0, j - 8), j):
            T, _ = Tij[(i, j - 1)]
            if T * (j - i) > SMAX:
                continue
            c = dp[i] + T * (j - i) + GROUP_FIX
            if c < dp[j]:
                dp[j] = c
                parent[j] = i
    groups = []
    j = BPC
    while j > 0:
        i = parent[j]
        groups.append((i, j - 1, Tij[(i, j - 1)][1]))
        j = i
    groups.reverse()

    # per-node bin assignment
    edge_bin = np.full(len(dst), -1, dtype=np.int8)
    gid_of_block = np.zeros(BPC, dtype=np.int64)
    for gi, (b0, b1, caps) in enumerate(groups):
        gid_of_block[b0:b1 + 1] = gi
    for gi, (b0, b1, caps) in enumerate(groups):
        node_mask = (nodeblk >= b0) & (nodeblk <= b1)
        nodes = np.flatnonzero(node_mask & (CC.sum(axis=1) > 0))
        C = CC[nodes]
        (eAE, oAO, eBE, oBO, fEe, fOo, fA), loads = _assign_bins(C, caps)
        capv = np.array(caps)
        bad = np.flatnonzero((loads > capv[None, :]).any(axis=1))
        # per-class -> bin counts [n, 6, 4]
        n = len(nodes)
        a = np.zeros((n, 6, 4), dtype=np.int64)
        a[:, 0, 2] = eAE
        a[:, 0, 0] = C[:, 0] - eAE
        a[:, 1, 3] = oAO
        a[:, 1, 0] = C[:, 1] - oAO
        a[:, 2, 2] = eBE
        a[:, 2, 1] = C[:, 2] - eBE
        a[:, 3, 3] = oBO
        a[:, 3, 1] = C[:, 3] - oBO
        a[:, 4, 2] = fEe
        a[:, 5, 3] = fOo
        restE = C[:, 4] - fEe
        restO = C[:, 5] - fOo
        fAe = np.minimum(restE, fA)
        a[:, 4, 0] = fAe
        a[:, 4, 1] = restE - fAe
        a[:, 5, 0] = fA - fAe
        a[:, 5, 1] = restO - (fA - fAe)
        for k in bad:
            a[k] = np.array(_assign_node_flow(C[k], caps))
        assert (a.sum(axis=1) <= capv[None, :]).all()
        assert (a.sum(axis=2) == C).all()
        # per (node, class): how many of that class go to each bin.
        # distribute actual edges: order edges of same (node, class)
        # arbitrarily, fill bins in option order
        CC_assign_nodes = nodes
        # store into a flat lookup: for edge e with node i, class c, its
        # rank r within (i, c) -> bin = smallest bin with cumulated cap
        # Build cumulative thresholds per (node, class): bins in fixed order
        # [A, B, E, O]
        thr = np.cumsum(a, axis=2)  # [n, 6, 4]
        # map from node id -> row in this group's arrays
        row_of = np.full(TBL_ROWS, -1, dtype=np.int64)
        row_of[nodes] = np.arange(n)
        em = node_mask[dpos]
        eidx = np.flatnonzero(em)
        enode = dpos[eidx]
        ecls = cls[eidx]
        # rank within (node, class)
        o2 = np.lexsort((ecls, enode))
        so = eidx[o2]
        key = dpos[so] * 6 + cls[so]
        chg = np.r_[True, key[1:] != key[:-1]]
        starts = np.flatnonzero(chg)
        gidv = np.cumsum(chg) - 1
        rank = np.arange(len(so)) - starts[gidv]
        r_ = row_of[dpos[so]]
        t = thr[r_, cls[so]]  # [m, 4]
        rk = rank[:, None]
        binv = (rk >= t).sum(axis=1)  # 0..3
        edge_bin[so] = binv.astype(np.int8)

    assert (edge_bin >= 0).all()

    # slot index within (node, bin)
    o3 = np.lexsort((edge_bin, dpos))
    so = o3
    key = dpos[so] * 4 + edge_bin[so]
    chg = np.r_[True, key[1:] != key[:-1]]
    starts = np.flatnonzero(chg)
    gidv = np.cumsum(chg) - 1
    slot = np.empty(len(so), dtype=np.int64)
    slot[so] = np.arange(len(so)) - starts[gidv]

    # dummy rows (present in every core stripe, [NPC, STRIDE))
    dmyA = NPC                                   # row 6250 < 32768
    dmyB_row = 3 * STRIDE + NPC                  # 25066 >= BBASE
    dmyB = dmyB_row - BBASE
    dmyE = NPC >> 1                              # row 6250 even
    dmyO = (NPC + 1) >> 1                        # row 6251 odd -> 3125
    assert NPC % 2 == 0
    dmy_vals = [dmyA, dmyB, dmyE, dmyO]

    # build per-core index tables: one flat [128, totcols] int16
    ecore = dpos // STRIDE
    eblk = blk
    elane = (dpos % STRIDE) % 128
    ebin = edge_bin.astype(np.int64)
    # index value per bin
    ival = np.where(ebin == 0, sp,
                    np.where(ebin == 1, sp - BBASE, sp >> 1))
    assert (ival >= 0).all() and (ival < 32768).all()

    offs = []          # per (group, bin): slot-column offset
    off = 0
    for (b0, b1, caps) in groups:
        gsz = b1 - b0 + 1
        bo = []
        for x in range(4):
            bo.append(off)
            off += caps[x] * gsz
        offs.append(bo)
    tot_slots = off

    idx_tables = []
    for c in range(CORES):
        flat = np.empty(128 * tot_slots, dtype=np.int64)
        # fill dummies per gather region
        for gi, (b0, b1, caps) in enumerate(groups):
            gsz = b1 - b0 + 1
            for x in range(4):
                o0 = offs[gi][x]
                flat[128 * o0:128 * (o0 + caps[x] * gsz)] = dmy_vals[x]
        m = ecore == c
        gi_e = gid_of_block[eblk[m]]
        b0_e = np.array([g[0] for g in groups])[gi_e]
        capmat = np.array([g[2] for g in groups])  # [ngroups, 4]
        Dx = capmat[gi_e, ebin[m]]
        off_e = np.array(offs)[gi_e, ebin[m]]
        col = off_e + (eblk[m] - b0_e) * Dx + slot[m]
        fpos = col * 128 + elane[m]
        assert len(np.unique(fpos)) == len(fpos)
        flat[fpos] = ival[m]
        wrapped = flat.reshape(-1, 16).T.astype(np.int16)  # [16, 8*tot]
        idx_tables.append(np.tile(wrapped, (8, 1)))        # [128, 8*tot]

    return dict(
        groups=groups, offs=offs, tot_slots=tot_slots,
        pos=pos, nodes_of_core=nodes_of_core,
        idx=idx_tables,
    )


# ---------------------------------------------------------------------------
# device program
# ---------------------------------------------------------------------------
def _build_program(groups, offs, tot_slots, has_b1):
    nc = bacc.Bacc("TRN2", target_bir_lowering=False, debug=False,
                   num_devices=CORES, num_swdge_queues=NQ)

    xTs = nc.dram_tensor("xTs", [128, STRIDE], F16, kind="ExternalInput")
    w1e = nc.dram_tensor("w1e", [128, W1N], F16, kind="ExternalInput")
    w2e = nc.dram_tensor("w2e", [L1H, W2N], F16, kind="ExternalInput")
    b1t = nc.dram_tensor("b1t", [128, L1H], F32, kind="ExternalInput")
    ident = nc.dram_tensor("ident", [128, 128], F32, kind="ExternalInput")
    idxt = nc.dram_tensor("idxt", [128, 8 * tot_slots], I16,
                          kind="ExternalInput")

    cc1 = nc.dram_tensor("cc1", [STRIDE, L1_ROW], F16)
    tbl1 = nc.dram_tensor("tbl1", [TBL_ROWS, L1_ROW], F16, addr_space="Shared")
    cc2 = nc.dram_tensor("cc2", [STRIDE, L2_ROW], F16)
    tbl2 = nc.dram_tensor("tbl2", [TBL_ROWS, L2_ROW], F16, addr_space="Shared")
    out = nc.dram_tensor("out", [STRIDE, OUT_DIM], F32, kind="ExternalOutput")

    with tile.TileContext(nc) as tc:
        with (
            tc.tile_pool(name="res", bufs=1) as res,
            tc.tile_pool(name="ps", bufs=2, space="PSUM") as psp,
            tc.tile_pool(name="sml", bufs=2) as sml,
        ):
            # ---- resident constants ----
            w1e_t = res.tile([128, W1N], F16, tag="w1e")
            nc.sync.dma_start(w1e_t[:], w1e.ap())
            w2e_t = res.tile([L1H, W2N], F16, tag="w2e")
            nc.sync.dma_start(w2e_t[:], w2e.ap())
            b1_t = res.tile([128, L1H], F32, tag="b1")
            nc.sync.dma_start(b1_t[:], b1t.ap())
            id_t = res.tile([128, 128], F32, tag="ident")
            nc.sync.dma_start(id_t[:], ident.ap())
            idx_t = res.tile([128, 8 * tot_slots], I16, tag="idx")
            nc.sync.dma_start(idx_t[:], idxt.ap())
            ad_own = res.tile([128, BPC * HEADS], F32, tag="adown")
            ad2_own = res.tile([128, BPC], F32, tag="ad2own")

            # dummy rows [NPC, STRIDE): h = 0, asrc = NEG_BIG
            pad_rows = STRIDE - NPC
            dmy1 = res.tile([pad_rows, L1_ROW], F16, tag="dmy1")
            nc.vector.memset(dmy1[:], 0.0)
            nc.vector.memset(dmy1[:, L1H:L1H + HEADS], NEG_BIG)
            nc.sync.dma_start(cc1.ap()[NPC:STRIDE, :], dmy1[:])
            dmy2 = res.tile([pad_rows, L2_ROW], F16, tag="dmy2")
            nc.vector.memset(dmy2[:], 0.0)
            nc.vector.memset(dmy2[:, OUT_DIM:OUT_DIM + 1], NEG_BIG)
            nc.sync.dma_start(cc2.ap()[NPC:STRIDE, :], dmy2[:])

            # ---- front end: this core's stripe of the fat-row table ----
            fe_ctx = tc.tile_pool(name="fe", bufs=3)
            fe = fe_ctx.__enter__()
            xts_t = res.tile([128, STRIDE], F16, tag="xts")
            nc.sync.dma_start(xts_t[:], xTs.ap())
            FCH = 7
            for i in range(3):
                f0 = fe.tile([128, FCH, L1_ROW], F16, tag="fat")
                nc.vector.memset(
                    f0[:, :, L1H + HEADS:L1H + 2 * HEADS], 1.0)
            for t0 in range(0, BPC, FCH):
                tn = min(FCH, BPC - t0)
                fat = fe.tile([128, FCH, L1_ROW], F16, tag="fat")
                for k in range(tn):
                    t = t0 + k
                    ps = psp.tile([128, W1N], F32, tag="feps")
                    nc.tensor.matmul(ps[:], xts_t[:, 128 * t:128 * (t + 1)],
                                     w1e_t[:], start=True, stop=True)
                    nc.scalar.activation(
                        fat[:, k, 0:L1H + HEADS], ps[:, 0:L1H + HEADS],
                        ACT.Copy)
                    nc.vector.tensor_copy(
                        ad_own[:, HEADS * t:HEADS * (t + 1)],
                        ps[:, L1H + HEADS:L1H + 2 * HEADS])
                nrows = min(128 * tn, NPC - 128 * t0)
                nfull = nrows // 128
                if nfull > 0:
                    nc.sync.dma_start(
                        cc1.ap()[128 * t0:128 * (t0 + nfull), :].rearrange(
                            "(t p) e -> p t e", p=128), fat[:, 0:nfull, :])
                rem = nrows - nfull * 128
                if rem > 0:
                    nc.sync.dma_start(
                        cc1.ap()[128 * (t0 + nfull):128 * (t0 + nfull) + rem,
                                 :], fat[0:rem, nfull, :])

            fe_ctx.__exit__(None, None, None)
            tc.strict_bb_all_engine_barrier()
            nc.gpsimd.collective_compute(
                "AllGather", AL.bypass,
                replica_groups=[list(range(CORES))],
                ins=[cc1.ap().opt()], outs=[tbl1.ap().opt()])
            tc.strict_bb_all_engine_barrier()

            # table views for the 4 bins
            t1A = tbl1.ap()[0:32768, :]
            t1B = tbl1.ap()[BBASE:TBL_ROWS, :]
            t1P = tbl1.ap().rearrange("(r two) e -> r (two e)", two=2)
            t1E = t1P[:, 0:L1_ROW]
            t1O = t1P[:, L1_ROW:2 * L1_ROW]

            def fire_gathers(gt, gi, caps, gsz, tA, tB, tE, tO, row):
                tabs = [tA, tB, tE, tO]
                steps = [row, row, 2 * row, 2 * row]
                qq = 0
                sofs = offs[gi][0]
                for x in range(4):
                    dn = caps[x] * gsz
                    if dn == 0:
                        continue
                    o0 = offs[gi][x] - sofs
                    nc.gpsimd.dma_gather(
                        gt[:, o0:o0 + dn, :], tabs[x],
                        idx_t[:, 8 * offs[gi][x]:8 * (offs[gi][x] + dn)],
                        128 * dn, 128 * dn, row,
                        elem_step=steps[x], single_packet=False,
                        queue_num=(2 * gi + qq) % NQ)
                    qq += 1

            def tree_fold(view, D):
                """view: [128, gsz, D, W]; fold slot dim -> slot 0."""
                Dt = 1 << (D.bit_length() - 1)
                if Dt == D and D > 1:
                    Dt >>= 1
                if D > Dt:
                    k = D - Dt
                    nc.vector.tensor_tensor(
                        view[:, :, 0:k, :], view[:, :, 0:k, :],
                        view[:, :, Dt:D, :], AL.add)
                k = Dt >> 1
                while k >= 1:
                    nc.vector.tensor_tensor(
                        view[:, :, 0:k, :], view[:, :, 0:k, :],
                        view[:, :, k:2 * k, :], AL.add)
                    k >>= 1

            # ---- layer 1 ----
            l1_gat_ctx = tc.tile_pool(name="gat1", bufs=2)
            gat = l1_gat_ctx.__enter__()
            l2f_pool_ctx = tc.tile_pool(name="l2f", bufs=2)
            l2fp = l2f_pool_ctx.__enter__()
            GMAX = max(b1 - b0 + 1 for b0, b1, _ in groups)
            for i in range(2):
                lf = l2fp.tile([128, GMAX, L2_ROW], F16, tag="l2f")
                nc.vector.memset(lf[:, :, OUT_DIM + 1:OUT_DIM + 2], 1.0)

            for gi, (b0, b1, caps) in enumerate(groups):
                gsz = b1 - b0 + 1
                S_g = sum(caps) * gsz
                gt = gat.tile([128, S_g, L1_ROW], F16, tag="g")
                fire_gathers(gt, gi, caps, gsz, t1A, t1B, t1E, t1O, L1_ROW)

                # z = asrc + adst  (per bin, block-broadcast)
                z = sml.tile([128, S_g, HEADS], F32, tag="z")
                adb = ad_own[:, HEADS * b0:HEADS * (b1 + 1)].rearrange(
                    "p (g h) -> p g h", g=gsz)
                sofs = offs[gi][0]
                for x in range(4):
                    if caps[x] == 0:
                        continue
                    o0 = offs[gi][x] - sofs
                    dn = caps[x] * gsz
                    nc.vector.tensor_tensor(
                        z[:, o0:o0 + dn, :].rearrange(
                            "p (g d) h -> p g d h", g=gsz),
                        gt[:, o0:o0 + dn, L1H:L1H + HEADS].rearrange(
                            "p (g d) h -> p g d h", g=gsz),
                        adb.unsqueeze(2).broadcast_to(
                            [128, gsz, caps[x], HEADS]), AL.add)
                z2 = sml.tile([128, S_g, HEADS], F32, tag="z2")
                nc.vector.scalar_tensor_tensor(
                    z2[:].rearrange("p a b -> p (a b)"),
                    z[:].rearrange("p a b -> p (a b)"), NEG_SLOPE,
                    z[:].rearrange("p a b -> p (a b)"),
                    op0=AL.mult, op1=AL.max)
                wb = sml.tile([128, S_g, HEADS], F16, tag="wb")
                nc.scalar.activation(
                    wb[:].rearrange("p a b -> p (a b)"),
                    z2[:].rearrange("p a b -> p (a b)"), ACT.Exp)

                # in-place weight multiply: h block and ones block
                nc.vector.tensor_tensor(
                    gt[:, :, 0:L1H].rearrange("p s (h c) -> p s h c", h=HEADS),
                    gt[:, :, 0:L1H].rearrange("p s (h c) -> p s h c", h=HEADS),
                    wb[:].unsqueeze(3).broadcast_to([128, S_g, HEADS, HID]),
                    AL.mult)
                nc.vector.tensor_tensor(
                    gt[:, :, L1H + HEADS:L1H + 2 * HEADS],
                    gt[:, :, L1H + HEADS:L1H + 2 * HEADS],
                    wb[:, :, :], AL.mult)

                # tree fold per bin, then cross-bin sum into f32
                W = L1H + 2 * HEADS    # 136: [wh | junk asrc | wsum]
                parts = []
                for x in range(4):
                    if caps[x] == 0:
                        continue
                    o0 = offs[gi][x] - sofs
                    v = gt[:, o0:o0 + caps[x] * gsz, 0:W].rearrange(
                        "p (g d) e -> p g d e", g=gsz)
                    tree_fold(v, caps[x])
                    parts.append(v[:, :, 0, :])
                r = sml.tile([128, GMAX, W], F32, tag="r")
                nc.vector.tensor_tensor(
                    r[:, 0:gsz, :], parts[0], parts[1], AL.add)
                for p_ in parts[2:]:
                    nc.vector.tensor_tensor(
                        r[:, 0:gsz, :], r[:, 0:gsz, :], p_, AL.add)

                rec = sml.tile([128, GMAX, HEADS], F32, tag="rec")
                nc.vector.reciprocal(
                    rec[:, 0:gsz, :],
                    r[:, 0:gsz, L1H + HEADS:L1H + 2 * HEADS])
                o1 = sml.tile([128, GMAX, L1H], F32, tag="o1")
                nc.vector.tensor_tensor(
                    o1[:, 0:gsz, :].rearrange("p g (h c) -> p g h c", h=HEADS),
                    r[:, 0:gsz, 0:L1H].rearrange("p g (h c) -> p g h c",
                                                 h=HEADS),
                    rec[:, 0:gsz, :].unsqueeze(3).broadcast_to(
                        [128, gsz, HEADS, HID]), AL.mult)
                if has_b1:
                    nc.vector.tensor_tensor(
                        o1[:, 0:gsz, :], o1[:, 0:gsz, :],
                        b1_t[:].unsqueeze(1).broadcast_to([128, gsz, L1H]),
                        AL.add)
                # elu(x) = max(x, exp(min(x, 0)) - 1)
                mn = sml.tile([128, GMAX, L1H], F32, tag="mn")
                nc.vector.tensor_scalar(
                    mn[:, 0:gsz, :], o1[:, 0:gsz, :], 0.0, None, op0=AL.min)
                e2 = sml.tile([128, GMAX, L1H], F32, tag="e2")
                nc.scalar.activation(
                    e2[:, 0:gsz, :], mn[:, 0:gsz, :], ACT.Exp)
                elu = sml.tile([128, GMAX, L1H], F32, tag="elu")
                nc.vector.scalar_tensor_tensor(
                    elu[:, 0:gsz, :], e2[:, 0:gsz, :], -1.0,
                    o1[:, 0:gsz, :], op0=AL.add, op1=AL.max)

                # layer-2 fat rows: h2' = elu^T @ W2ext per block
                l2fat = l2fp.tile([128, GMAX, L2_ROW], F16, tag="l2f")
                for k in range(gsz):
                    b = b0 + k
                    tp = psp.tile([128, 128], F32, tag="tp")
                    nc.tensor.transpose(tp[:], elu[:, k, :], id_t[:])
                    eluT = sml.tile([128, 128], F16, tag="eluT")
                    nc.scalar.activation(eluT[:], tp[:], ACT.Copy)
                    h2p = psp.tile([128, W2N], F32, tag="h2p")
                    nc.tensor.matmul(h2p[:], eluT[:], w2e_t[:],
                                     start=True, stop=True)
                    nc.scalar.activation(
                        l2fat[:, k, 0:OUT_DIM + 1], h2p[:, 0:OUT_DIM + 1],
                        ACT.Copy)
                    nc.vector.tensor_copy(
                        ad2_own[:, b:b + 1], h2p[:, OUT_DIM + 1:OUT_DIM + 2])
                nrows = min(128 * gsz, NPC - 128 * b0)
                nfull = nrows // 128
                if nfull > 0:
                    nc.sync.dma_start(
                        cc2.ap()[128 * b0:128 * (b0 + nfull), :].rearrange(
                            "(t p) e -> p t e", p=128), l2fat[:, 0:nfull, :])
                rem = nrows - nfull * 128
                if rem > 0:
                    nc.sync.dma_start(
                        cc2.ap()[128 * (b0 + nfull):128 * (b0 + nfull) + rem,
                                 :], l2fat[0:rem, nfull, :])

            l2f_pool_ctx.__exit__(None, None, None)
            l1_gat_ctx.__exit__(None, None, None)
            tc.strict_bb_all_engine_barrier()
            nc.gpsimd.collective_compute(
                "AllGather", AL.bypass,
                replica_groups=[list(range(CORES))],
                ins=[cc2.ap().opt()], outs=[tbl2.ap().opt()])
            tc.strict_bb_all_engine_barrier()

            # ---- layer 2 ----
            t2A = tbl2.ap()[0:32768, :]
            t2B = tbl2.ap()[BBASE:TBL_ROWS, :]
            t2P = tbl2.ap().rearrange("(r two) e -> r (two e)", two=2)
            t2E = t2P[:, 0:L2_ROW]
            t2O = t2P[:, L2_ROW:2 * L2_ROW]
            W2R = OUT_DIM + 2      # reduce width: [wh2 | junk a2s | wsum]

            l2_gat_ctx = tc.tile_pool(name="gat2", bufs=2)
            gat = l2_gat_ctx.__enter__()
            for gi, (b0, b1, caps) in enumerate(groups):
                gsz = b1 - b0 + 1
                S_g = sum(caps) * gsz
                g2 = gat.tile([128, S_g, L2_ROW], F16, tag="g2")
                fire_gathers(g2, gi, caps, gsz, t2A, t2B, t2E, t2O, L2_ROW)

                z = sml.tile([128, S_g], F32, tag="z2l")
                ad2b = ad2_own[:, b0:b1 + 1]
                sofs = offs[gi][0]
                for x in range(4):
                    if caps[x] == 0:
                        continue
                    o0 = offs[gi][x] - sofs
                    nc.vector.tensor_tensor(
                        z[:, o0:o0 + caps[x] * gsz].rearrange(
                            "p (g d) -> p g d", g=gsz),
                        g2[:, o0:o0 + caps[x] * gsz, OUT_DIM].rearrange(
                            "p (g d) -> p g d", g=gsz),
                        ad2b.unsqueeze(2).broadcast_to([128, gsz, caps[x]]),
                        AL.add)
                z2 = sml.tile([128, S_g], F32, tag="z2l2")
                nc.vector.scalar_tensor_tensor(
                    z2[:, :], z[:, :], NEG_SLOPE, z[:, :],
                    op0=AL.mult, op1=AL.max)
                w2t = sml.tile([128, S_g], F16, tag="w2t")
                nc.scalar.activation(w2t[:, :], z2[:, :], ACT.Exp)

                nc.vector.tensor_tensor(
                    g2[:, :, 0:OUT_DIM], g2[:, :, 0:OUT_DIM],
                    w2t[:].unsqueeze(2).broadcast_to([128, S_g, OUT_DIM]),
                    AL.mult)
                nc.vector.tensor_tensor(
                    g2[:, :, OUT_DIM + 1:OUT_DIM + 2],
                    g2[:, :, OUT_DIM + 1:OUT_DIM + 2],
                    w2t[:].unsqueeze(2), AL.mult)

                parts = []
                for x in range(4):
                    if caps[x] == 0:
                        continue
                    o0 = offs[gi][x] - sofs
                    v = g2[:, o0:o0 + caps[x] * gsz, 0:W2R].rearrange(
                        "p (g d) e -> p g d e", g=gsz)
                    tree_fold(v, caps[x])
                    parts.append(v[:, :, 0, :])
                r = sml.tile([128, GMAX, W2R], F32, tag="r2")
                nc.vector.tensor_tensor(
                    r[:, 0:gsz, :], parts[0], parts[1], AL.add)
                for p_ in parts[2:]:
                    nc.vector.tensor_tensor(
                        r[:, 0:gsz, :], r[:, 0:gsz, :], p_, AL.add)

                rec = sml.tile([128, GMAX], F32, tag="rec2")
                nc.vector.reciprocal(
                    rec[:, 0:gsz], r[:, 0:gsz, OUT_DIM + 1])
                o2 = sml.tile([128, GMAX, OUT_DIM], F32, tag="o2")
                nc.vector.tensor_tensor(
                    o2[:, 0:gsz, :], r[:, 0:gsz, 0:OUT_DIM],
                    rec[:, 0:gsz].unsqueeze(2).broadcast_to(
                        [128, gsz, OUT_DIM]), AL.mult)
                nrows = min(128 * gsz, NPC - 128 * b0)
                nfull = nrows // 128
                if nfull > 0:
                    nc.sync.dma_start(
                        out.ap()[128 * b0:128 * (b0 + nfull), :].rearrange(
                            "(t p) e -> p t e", p=128), o2[:, 0:nfull, :])
                rem = nrows - nfull * 128
                if rem > 0:
                    nc.sync.dma_start(
                        out.ap()[128 * (b0 + nfull):128 * (b0 + nfull) + rem,
                                 :], o2[0:rem, nfull, :])

            l2_gat_ctx.__exit__(None, None, None)

    nc.compile()
    return nc


# ---------------------------------------------------------------------------
# weight prep + end-to-end run
# ---------------------------------------------------------------------------
def _run(x, edge_index, W1, a1_src, a1_dst, b1, W2, a2_src, a2_dst, b2,
         trace=False, n_nodes=None, bpc=None):
    x = np.asarray(x, dtype=np.float32)
    edge_index = np.asarray(edge_index)

    g = _prep_graph(edge_index, N)

    has_b1 = bool(np.abs(np.asarray(b1)).max() > 0)
    key = (5, has_b1, tuple((b0, b1, tuple(c)) for b0, b1, c in g["groups"]))
    if key in _CACHE:
        nc = _CACHE[key]
    else:
        nc = _build_program(g["groups"], g["offs"], g["tot_slots"], has_b1)
        _CACHE[key] = nc

    W1 = np.asarray(W1, np.float32)
    W2 = np.asarray(W2, np.float32)
    w1s = np.stack([W1[:, h * HID:(h + 1) * HID]
                    @ np.asarray(a1_src, np.float32)[h]
                    for h in range(HEADS)], axis=1)
    w1d = np.stack([W1[:, h * HID:(h + 1) * HID]
                    @ np.asarray(a1_dst, np.float32)[h]
                    for h in range(HEADS)], axis=1)
    w1e_np = np.concatenate([W1, w1s, w1d], axis=1)
    w2s = (W2 @ np.asarray(a2_src, np.float32)[0])[:, None]
    w2d = (W2 @ np.asarray(a2_dst, np.float32)[0])[:, None]
    w2e_np = np.concatenate([W2, w2s, w2d], axis=1)

    xT = np.zeros((IN_DIM, TBL_ROWS), dtype=np.float32)
    xT[:, g["pos"]] = x.T

    common = {
        "w1e": w1e_np.astype(np.float16),
        "w2e": w2e_np.astype(np.float16),
        "b1t": np.tile(np.asarray(b1, np.float32)[None, :], (128, 1)),
        "ident": np.eye(128, dtype=np.float32),
    }
    in_maps = []
    for c in range(CORES):
        in_maps.append({
            **common,
            "xTs": xT[:, c * STRIDE:(c + 1) * STRIDE].astype(np.float16),
            "idxt": g["idx"][c],
        })

    res = run_bass_kernel_spmd(nc, in_maps, list(range(CORES)), trace=trace)

    out_full = np.empty((N, OUT_DIM), dtype=np.float32)
    for c in range(CORES):
        out_full[g["nodes_of_core"][c]] = res.results[c]["out"][0:NPC]
    out_full += np.asarray(b2, np.float32)[None, :]
    return out_full, res


def kernel(x, edge_index, W1, a1_src, a1_dst, b1, W2, a2_src, a2_dst, b2):
    out, _ = _run(x, edge_index, W1, a1_src, a1_dst, b1, W2, a2_src, a2_dst,
                  b2)
    return out


# revision 7
# speedup vs baseline: 1.3467x; 1.0460x over previous
"""Two-layer GAT kernel for 8 Trainium2 NeuronCores — v3.

v3 strategy (4-bin gathers, grouped blocks, in-place fp16 slot reduce):
  * Nodes degree-sorted, dealt round-robin to 8 cores; each core aggregates
    messages for its 6250 nodes (dst-sharded, no cross-core reduce).
  * Sharded front end computes each core's stripe of a fat-row table
    (fp16, 512B rows: [h(128) | asrc(4) | ones(4) | junk]); AllGather
    replicates it.  Layer 2 repeats with 256B fp16 rows
    [h2(32) | a2s | one | junk].
  * Per-edge rows fetched with dma_gather (SWDGE, int16 indices < 32768).
    FOUR index bins per block group make every edge 2-3 way assignable,
    minimizing the per-block slot maxima:
      A: rows [0, 32768)        stride 512B
      B: rows [17408, 50176)    stride 512B
      E: even rows (any)        stride 1024B (idx = row>>1)
      O: odd rows (any)         stride 1024B from +512B base
    Bin capacities per group are chosen by an exact small LP (Hall
    feasibility over the 16 bin subsets); blocks are grouped by a DP that
    trades gather fixed cost against slot padding.
  * Gathers are grouped (one gather per bin per group of 3-6 blocks) to
    amortize the ~1.2us SWDGE fixed cost, rotating 4 SWDGE queues.
  * Attention weights are multiplied into the gathered rows IN PLACE
    (fp16, 2x DVE rate), then a pairwise tree reduce folds slots per
    block; static "ones" columns produce the softmax denominator in the
    same reduce.  Padding slots point at dummy rows with asrc = -30000
    (exp == 0 in fp16).
"""

import sys

sys.path.insert(0, "/opt/trn_rl_repo")

import numpy as np

import concourse.bacc as bacc
import concourse.bass as bass
import concourse.mybir as mybir
import concourse.tile as tile
from concourse.bass_utils import run_bass_kernel_spmd

F32 = mybir.dt.float32
F16 = mybir.dt.float16
I16 = mybir.dt.int16
AL = mybir.AluOpType
ACT = mybir.ActivationFunctionType

CORES = 8
NEG_SLOPE = 0.2
NEG_BIG = -30000.0          # fp16-safe; exp(leaky(-30000)) == 0

# problem constants (nn_GAT_35296041238878)
N = 50000
IN_DIM = 128
HID = 32
HEADS = 4
OUT_DIM = 32

BPC = 49
STRIDE = BPC * 128           # 6272 table rows per core
TBL_ROWS = CORES * STRIDE    # 50176
BBASE = TBL_ROWS - 32768     # 17408, window-B base
NPC = N // CORES             # 6250 real nodes per core

# layer-1 row (fp16, 512B): [h(0:128) | asrc(128:132) | ones(132:136) | junk]
L1_ROW = 256
L1H = HEADS * HID            # 128
W1N = L1H + 2 * HEADS        # 136 matmul cols [h | asrc | adst]
# layer-2 row (fp16, 256B): [h2(0:32) | a2s(32) | one(33) | junk]
L2_ROW = 128
W2N = OUT_DIM + 2            # 34 matmul cols [h2 | a2s | a2d]

NQ = 4                       # SWDGE queues
SMAX = 100                   # max slots per group tile (SBUF bound)
GROUP_FIX = 12               # DP: group fixed cost in slot units

_CACHE = {}

# ---------------------------------------------------------------------------
# Tile's DMASW lane round-robin is not SWDGE-queue-aware: partition the 8
# lanes so queue q uses lanes [q*2, q*2+2).
import concourse.tile_sem_assignment as _tsa


def _queue_aware_assign_tick(self, inst):
    q = getattr(inst, "queue_num", None)
    if q is not None and isinstance(inst, _tsa.DMAInst) \
            and inst.engine == _tsa.mybir.EngineType.Pool:
        if not hasattr(self, "_q_lane_ctr"):
            self._q_lane_ctr = {}
        ctr = self._q_lane_ctr.get(q, 0)
        self._q_lane_ctr[q] = ctr + 1
        lanes = max(1, self.swdge_sem_count // NQ)
        self.next_sw_dma_idx = (q % NQ) * lanes + (ctr % lanes)
    return _tsa.TileClockTick._orig_assign_tick(self, inst)


if not hasattr(_tsa.TileClockTick, "_orig_assign_tick"):
    _tsa.TileClockTick._orig_assign_tick = _tsa.TileClockTick._assign_tick
    _tsa.TileClockTick._assign_tick = _queue_aware_assign_tick


# ---------------------------------------------------------------------------
# host-side graph preprocessing
# ---------------------------------------------------------------------------
_COPT = np.array([1 | 4, 1 | 8, 2 | 4, 2 | 8, 1 | 2 | 4, 1 | 2 | 8])
_SUBSEL = [[(int(_COPT[c]) & ~S) == 0 for c in range(6)] for S in range(16)]


def _opt_caps(M):
    """Min DA+DB+DE+DO s.t. sum over S-subset >= M[S] for all 16 subsets."""
    for T in range(int(M[15]), int(M[15]) + 16):
        for DA in range(int(M[1]), T + 1):
            for DB in range(int(M[2]), T - DA + 1):
                for DE in range(int(M[4]), T - DA - DB + 1):
                    DO = T - DA - DB - DE
                    if DO < M[8]:
                        continue
                    D = (DA, DB, DE, DO)
                    ok = True
                    for S in range(16):
                        cap = 0
                        for i in range(4):
                            if S >> i & 1:
                                cap += D[i]
                        if cap < M[S]:
                            ok = False
                            break
                    if ok:
                        return T, D
    raise RuntimeError("cap search failed")


def _assign_bins(C, caps):
    """Per-node edge->bin counts. C: [n,6] class counts (AE,AO,BE,BO,FE,FO).
    Returns [n,6] arrays: for each class, how many go to the PARITY bin
    (classes 0-3) / how many flex go to A (classes 4,5 -> (toE/toO, toA)).
    Output: eAE,oAO,eBE,oBO (fixed classes sent to parity), fEe,fOo
    (flex sent to parity), fA (flex sent to A; rest of flex goes to B)."""
    DA, DB, DE, DO = caps
    cAE, cAO, cBE, cBO, cFE, cFO = [C[:, i].astype(np.int64) for i in range(6)]
    needA = np.maximum(0, cAE + cAO - DA)
    needB = np.maximum(0, cBE + cBO - DB)
    eAE = np.minimum(cAE, needA)
    oAO = needA - eAE
    eBE = np.minimum(cBE, needB)
    oBO = needB - eBE
    # parity cap overflow: shift A-relief / B-relief between E and O
    for _ in range(2):
        xsE = np.maximum(0, (eAE + eBE) - DE)
        # shift AE->AO
        s1 = np.minimum(np.minimum(eAE, cAO - oAO), xsE)
        eAE -= s1
        oAO += s1
        xsE -= s1
        # shift BE->BO
        s2 = np.minimum(np.minimum(eBE, cBO - oBO), xsE)
        eBE -= s2
        oBO += s2
        xsO = np.maximum(0, (oAO + oBO) - DO)
        s3 = np.minimum(np.minimum(oAO, cAE - eAE), xsO)
        oAO -= s3
        eAE += s3
        xsO -= s3
        s4 = np.minimum(np.minimum(oBO, cBE - eBE), xsO)
        oBO -= s4
        eBE += s4
    # flex placement: parity first, then A, then B
    slackE = DE - (eAE + eBE)
    slackO = DO - (oAO + oBO)
    slackA = DA - (cAE - eAE + cAO - oAO)
    slackB = DB - (cBE - eBE + cBO - oBO)
    fEe = np.minimum(cFE, np.maximum(0, slackE))
    fOo = np.minimum(cFO, np.maximum(0, slackO))
    rest = (cFE - fEe) + (cFO - fOo)
    fA = np.minimum(rest, np.maximum(0, slackA))
    # remainder goes to B implicitly
    loads = np.stack([
        cAE - eAE + cAO - oAO + fA,
        cBE - eBE + cBO - oBO + (rest - fA),
        eAE + eBE + fEe,
        oAO + oBO + fOo,
    ], axis=1)
    return (eAE, oAO, eBE, oBO, fEe, fOo, fA), loads


def _assign_node_flow(counts, caps):
    """Exact per-node assignment by tiny augmenting flow. counts: [6]."""
    DA, DB, DE, DO = caps
    cap = [DA, DB, DE, DO]
    load = [0, 0, 0, 0]
    opts = [(0, 2), (0, 3), (1, 2), (1, 3), (0, 1, 2), (0, 1, 3)]
    assign = [[0] * 4 for _ in range(6)]
    for c in range(6):
        for _ in range(int(counts[c])):
            placed = False
            for b in opts[c]:
                if load[b] < cap[b]:
                    load[b] += 1
                    assign[c][b] += 1
                    placed = True
                    break
            if placed:
                continue
            # augment: move one edge of some class c2 from bin b (an option
            # of c) to another bin b2 of c2 with room
            done = False
            for b in opts[c]:
                for c2 in range(6):
                    if done or assign[c2][b] == 0:
                        continue
                    for b2 in opts[c2]:
                        if b2 != b and load[b2] < cap[b2]:
                            assign[c2][b] -= 1
                            assign[c2][b2] += 1
                            load[b2] += 1
                            assign[c][b] += 1
                            done = True
                            break
                if done:
                    break
            if not done:
                raise RuntimeError("node assignment infeasible")
    return assign


def _prep_graph(edge_index, n_nodes):
    src = np.concatenate([edge_index[0], np.arange(n_nodes)]).astype(np.int64)
    dst = np.concatenate([edge_index[1], np.arange(n_nodes)]).astype(np.int64)

    deg = np.bincount(dst, minlength=n_nodes)
    order = np.argsort(-deg, kind="stable")
    pos = np.empty(n_nodes, dtype=np.int64)
    ranks = np.arange(n_nodes)
    pos[order] = (ranks % CORES) * STRIDE + ranks // CORES
    nodes_of_core = [order[c::CORES] for c in range(CORES)]

    dpos = pos[dst]
    sp = pos[src]
    blk = (dpos % STRIDE) // 128

    Acap = sp < 32768
    Bcap = sp >= BBASE
    par = (sp & 1).astype(np.int64)
    flex = Acap & Bcap
    onlyA = Acap & ~Bcap
    cls = np.where(flex, 4 + par, np.where(onlyA, 0 + par, 2 + par))

    CC = np.zeros((TBL_ROWS, 6), dtype=np.int32)
    np.add.at(CC, (dpos, cls), 1)

    nodeblk = (np.arange(TBL_ROWS) % STRIDE) // 128
    Mb = np.zeros((BPC, 16), dtype=np.int64)
    for S in range(1, 16):
        sel = _SUBSEL[S]
        if any(sel):
            dem = CC[:, sel].sum(axis=1)
            for b in range(BPC):
                Mb[b, S] = dem[nodeblk == b].max()

    # caps for every candidate contiguous block group of size <= 8
    Tij = {}
    for i in range(BPC):
        M = np.zeros(16, dtype=np.int64)
        for j in range(i, min(i + 8, BPC)):
            M = np.maximum(M, Mb[j])
            Tij[(i, j)] = _opt_caps(M)

    # DP over group boundaries
    INF = 1 << 30
    dp = [INF] * (BPC + 1)
    dp[0] = 0
    parent = [0] * (BPC + 1)
    for j in range(1, BPC + 1):
        for i in range(max(0, j - 8), j):
            T, _ = Tij[(i, j - 1)]
            if T * (j - i) > SMAX:
                continue
            c = dp[i] + T * (j - i) + GROUP_FIX
            if c < dp[j]:
                dp[j] = c
                parent[j] = i
    groups = []
    j = BPC
    while j > 0:
        i = parent[j]
        groups.append((i, j - 1, Tij[(i, j - 1)][1]))
        j = i
    groups.reverse()

    # per-node bin assignment
    edge_bin = np.full(len(dst), -1, dtype=np.int8)
    gid_of_block = np.zeros(BPC, dtype=np.int64)
    for gi, (b0, b1, caps) in enumerate(groups):
        gid_of_block[b0:b1 + 1] = gi
    for gi, (b0, b1, caps) in enumerate(groups):
        node_mask = (nodeblk >= b0) & (nodeblk <= b1)
        nodes = np.flatnonzero(node_mask & (CC.sum(axis=1) > 0))
        C = CC[nodes]
        (eAE, oAO, eBE, oBO, fEe, fOo, fA), loads = _assign_bins(C, caps)
        capv = np.array(caps)
        bad = np.flatnonzero((loads > capv[None, :]).any(axis=1))
        # per-class -> bin counts [n, 6, 4]
        n = len(nodes)
        a = np.zeros((n, 6, 4), dtype=np.int64)
        a[:, 0, 2] = eAE
        a[:, 0, 0] = C[:, 0] - eAE
        a[:, 1, 3] = oAO
        a[:, 1, 0] = C[:, 1] - oAO
        a[:, 2, 2] = eBE
        a[:, 2, 1] = C[:, 2] - eBE
        a[:, 3, 3] = oBO
        a[:, 3, 1] = C[:, 3] - oBO
        a[:, 4, 2] = fEe
        a[:, 5, 3] = fOo
        restE = C[:, 4] - fEe
        restO = C[:, 5] - fOo
        fAe = np.minimum(restE, fA)
        a[:, 4, 0] = fAe
        a[:, 4, 1] = restE - fAe
        a[:, 5, 0] = fA - fAe
        a[:, 5, 1] = restO - (fA - fAe)
        for k in bad:
            a[k] = np.array(_assign_node_flow(C[k], caps))
        assert (a.sum(axis=1) <= capv[None, :]).all()
        assert (a.sum(axis=2) == C).all()
        # per (node, class): how many of that class go to each bin.
        # distribute actual edges: order edges of same (node, class)
        # arbitrarily, fill bins in option order
        CC_assign_nodes = nodes
        # store into a flat lookup: for edge e with node i, class c, its
        # rank r within (i, c) -> bin = smallest bin with cumulated cap
        # Build cumulative thresholds per (node, class): bins in fixed order
        # [A, B, E, O]
        thr = np.cumsum(a, axis=2)  # [n, 6, 4]
        # map from node id -> row in this group's arrays
        row_of = np.full(TBL_ROWS, -1, dtype=np.int64)
        row_of[nodes] = np.arange(n)
        em = node_mask[dpos]
        eidx = np.flatnonzero(em)
        enode = dpos[eidx]
        ecls = cls[eidx]
        # rank within (node, class)
        o2 = np.lexsort((ecls, enode))
        so = eidx[o2]
        key = dpos[so] * 6 + cls[so]
        chg = np.r_[True, key[1:] != key[:-1]]
        starts = np.flatnonzero(chg)
        gidv = np.cumsum(chg) - 1
        rank = np.arange(len(so)) - starts[gidv]
        r_ = row_of[dpos[so]]
        t = thr[r_, cls[so]]  # [m, 4]
        rk = rank[:, None]
        binv = (rk >= t).sum(axis=1)  # 0..3
        edge_bin[so] = binv.astype(np.int8)

    assert (edge_bin >= 0).all()

    # slot index within (node, bin)
    o3 = np.lexsort((edge_bin, dpos))
    so = o3
    key = dpos[so] * 4 + edge_bin[so]
    chg = np.r_[True, key[1:] != key[:-1]]
    starts = np.flatnonzero(chg)
    gidv = np.cumsum(chg) - 1
    slot = np.empty(len(so), dtype=np.int64)
    slot[so] = np.arange(len(so)) - starts[gidv]

    # dummy rows (present in every core stripe, [NPC, STRIDE))
    dmyA = NPC                                   # row 6250 < 32768
    dmyB_row = 3 * STRIDE + NPC                  # 25066 >= BBASE
    dmyB = dmyB_row - BBASE
    dmyE = NPC >> 1                              # row 6250 even
    dmyO = (NPC + 1) >> 1                        # row 6251 odd -> 3125
    assert NPC % 2 == 0
    dmy_vals = [dmyA, dmyB, dmyE, dmyO]

    # build per-core index tables: one flat [128, totcols] int16
    ecore = dpos // STRIDE
    eblk = blk
    elane = (dpos % STRIDE) % 128
    ebin = edge_bin.astype(np.int64)
    # index value per bin
    ival = np.where(ebin == 0, sp,
                    np.where(ebin == 1, sp - BBASE, sp >> 1))
    assert (ival >= 0).all() and (ival < 32768).all()

    offs = []          # per (group, bin): slot-column offset
    off = 0
    for (b0, b1, caps) in groups:
        gsz = b1 - b0 + 1
        bo = []
        for x in range(4):
            bo.append(off)
            off += caps[x] * gsz
        offs.append(bo)
    tot_slots = off

    idx_tables = []
    for c in range(CORES):
        flat = np.empty(128 * tot_slots, dtype=np.int64)
        # fill dummies per gather region
        for gi, (b0, b1, caps) in enumerate(groups):
            gsz = b1 - b0 + 1
            for x in range(4):
                o0 = offs[gi][x]
                flat[128 * o0:128 * (o0 + caps[x] * gsz)] = dmy_vals[x]
        m = ecore == c
        gi_e = gid_of_block[eblk[m]]
        b0_e = np.array([g[0] for g in groups])[gi_e]
        gsz_e = np.array([g[1] - g[0] + 1 for g in groups])[gi_e]
        capmat = np.array([g[2] for g in groups])  # [ngroups, 4]
        Dx = capmat[gi_e, ebin[m]]
        off_e = np.array(offs)[gi_e, ebin[m]]
        col = off_e + slot[m] * gsz_e + (eblk[m] - b0_e)
        fpos = col * 128 + elane[m]
        assert len(np.unique(fpos)) == len(fpos)
        flat[fpos] = ival[m]
        wrapped = flat.reshape(-1, 16).T.astype(np.int16)  # [16, 8*tot]
        idx_tables.append(np.tile(wrapped, (8, 1)))        # [128, 8*tot]

    return dict(
        groups=groups, offs=offs, tot_slots=tot_slots,
        pos=pos, nodes_of_core=nodes_of_core,
        idx=idx_tables,
    )


# ---------------------------------------------------------------------------
# device program
# ---------------------------------------------------------------------------
def _build_program(groups, offs, tot_slots, has_b1):
    nc = bacc.Bacc("TRN2", target_bir_lowering=False, debug=False,
                   num_devices=CORES, num_swdge_queues=NQ)

    xTs = nc.dram_tensor("xTs", [128, STRIDE], F16, kind="ExternalInput")
    w1e = nc.dram_tensor("w1e", [128, W1N], F16, kind="ExternalInput")
    w2e = nc.dram_tensor("w2e", [L1H, W2N], F16, kind="ExternalInput")
    b1t = nc.dram_tensor("b1t", [128, L1H], F32, kind="ExternalInput")
    ident = nc.dram_tensor("ident", [128, 128], F32, kind="ExternalInput")
    idxt = nc.dram_tensor("idxt", [128, 8 * tot_slots], I16,
                          kind="ExternalInput")

    cc1 = nc.dram_tensor("cc1", [STRIDE, L1_ROW], F16)
    tbl1 = nc.dram_tensor("tbl1", [TBL_ROWS, L1_ROW], F16, addr_space="Shared")
    cc2 = nc.dram_tensor("cc2", [STRIDE, L2_ROW], F16)
    tbl2 = nc.dram_tensor("tbl2", [TBL_ROWS, L2_ROW], F16, addr_space="Shared")
    out = nc.dram_tensor("out", [STRIDE, OUT_DIM], F32, kind="ExternalOutput")

    with tile.TileContext(nc) as tc:
        with (
            tc.tile_pool(name="res", bufs=1) as res,
            tc.tile_pool(name="ps", bufs=2, space="PSUM") as psp,
            tc.tile_pool(name="sml", bufs=2) as sml,
        ):
            # ---- resident constants ----
            w1e_t = res.tile([128, W1N], F16, tag="w1e")
            nc.sync.dma_start(w1e_t[:], w1e.ap())
            w2e_t = res.tile([L1H, W2N], F16, tag="w2e")
            nc.sync.dma_start(w2e_t[:], w2e.ap())
            b1_t = res.tile([128, L1H], F32, tag="b1")
            nc.sync.dma_start(b1_t[:], b1t.ap())
            id_t = res.tile([128, 128], F32, tag="ident")
            nc.sync.dma_start(id_t[:], ident.ap())
            idx_t = res.tile([128, 8 * tot_slots], I16, tag="idx")
            nc.sync.dma_start(idx_t[:], idxt.ap())
            ad_own = res.tile([128, BPC * HEADS], F16, tag="adown")
            ad2_own = res.tile([128, BPC], F16, tag="ad2own")

            # dummy rows [NPC, STRIDE): h = 0, asrc = NEG_BIG
            pad_rows = STRIDE - NPC
            dmy1 = res.tile([pad_rows, L1_ROW], F16, tag="dmy1")
            nc.vector.memset(dmy1[:], 0.0)
            nc.vector.memset(dmy1[:, L1H:L1H + HEADS], NEG_BIG)
            nc.sync.dma_start(cc1.ap()[NPC:STRIDE, :], dmy1[:])
            dmy2 = res.tile([pad_rows, L2_ROW], F16, tag="dmy2")
            nc.vector.memset(dmy2[:], 0.0)
            nc.vector.memset(dmy2[:, OUT_DIM:OUT_DIM + 1], NEG_BIG)
            nc.sync.dma_start(cc2.ap()[NPC:STRIDE, :], dmy2[:])

            # ---- front end: this core's stripe of the fat-row table ----
            fe_ctx = tc.tile_pool(name="fe", bufs=3)
            fe = fe_ctx.__enter__()
            xts_t = res.tile([128, STRIDE], F16, tag="xts")
            nc.sync.dma_start(xts_t[:], xTs.ap())
            FCH = 7
            for i in range(3):
                f0 = fe.tile([128, FCH, L1_ROW], F16, tag="fat")
                nc.vector.memset(
                    f0[:, :, L1H + HEADS:L1H + 2 * HEADS], 1.0)
            for t0 in range(0, BPC, FCH):
                tn = min(FCH, BPC - t0)
                fat = fe.tile([128, FCH, L1_ROW], F16, tag="fat")
                for k in range(tn):
                    t = t0 + k
                    ps = psp.tile([128, W1N], F32, tag="feps")
                    nc.tensor.matmul(ps[:], xts_t[:, 128 * t:128 * (t + 1)],
                                     w1e_t[:], start=True, stop=True)
                    nc.scalar.activation(
                        fat[:, k, 0:L1H + HEADS], ps[:, 0:L1H + HEADS],
                        ACT.Copy)
                    nc.vector.tensor_copy(
                        ad_own[:, HEADS * t:HEADS * (t + 1)],
                        ps[:, L1H + HEADS:L1H + 2 * HEADS])
                nrows = min(128 * tn, NPC - 128 * t0)
                nfull = nrows // 128
                if nfull > 0:
                    nc.sync.dma_start(
                        cc1.ap()[128 * t0:128 * (t0 + nfull), :].rearrange(
                            "(t p) e -> p t e", p=128), fat[:, 0:nfull, :])
                rem = nrows - nfull * 128
                if rem > 0:
                    nc.sync.dma_start(
                        cc1.ap()[128 * (t0 + nfull):128 * (t0 + nfull) + rem,
                                 :], fat[0:rem, nfull, :])

            fe_ctx.__exit__(None, None, None)
            tc.strict_bb_all_engine_barrier()
            nc.gpsimd.collective_compute(
                "AllGather", AL.bypass,
                replica_groups=[list(range(CORES))],
                ins=[cc1.ap().opt()], outs=[tbl1.ap().opt()])
            tc.strict_bb_all_engine_barrier()

            # table views for the 4 bins
            t1A = tbl1.ap()[0:32768, :]
            t1B = tbl1.ap()[BBASE:TBL_ROWS, :]
            t1P = tbl1.ap().rearrange("(r two) e -> r (two e)", two=2)
            t1E = t1P[:, 0:L1_ROW]
            t1O = t1P[:, L1_ROW:2 * L1_ROW]

            def fire_gathers(gt, gi, caps, gsz, tA, tB, tE, tO, row):
                tabs = [tA, tB, tE, tO]
                steps = [row, row, 2 * row, 2 * row]
                qq = 0
                sofs = offs[gi][0]
                for x in range(4):
                    dn = caps[x] * gsz
                    if dn == 0:
                        continue
                    o0 = offs[gi][x] - sofs
                    nc.gpsimd.dma_gather(
                        gt[:, o0:o0 + dn, :], tabs[x],
                        idx_t[:, 8 * offs[gi][x]:8 * (offs[gi][x] + dn)],
                        128 * dn, 128 * dn, row,
                        elem_step=steps[x], single_packet=False,
                        queue_num=(2 * gi + qq) % NQ)
                    qq += 1

            def tree_fold(buf, o0, D, gsz, W):
                """Slot-major bin region: columns [o0, o0+D*gsz), fold the
                slot dim (outer) -> partial sums in columns [o0, o0+gsz)."""
                Dt = 1 << (D.bit_length() - 1)
                if Dt == D and D > 1:
                    Dt >>= 1
                if D > Dt:
                    k = D - Dt
                    nc.vector.tensor_tensor(
                        buf[:, o0:o0 + k * gsz, 0:W],
                        buf[:, o0:o0 + k * gsz, 0:W],
                        buf[:, o0 + Dt * gsz:o0 + D * gsz, 0:W], AL.add)
                k = Dt >> 1
                while k >= 1:
                    nc.vector.tensor_tensor(
                        buf[:, o0:o0 + k * gsz, 0:W],
                        buf[:, o0:o0 + k * gsz, 0:W],
                        buf[:, o0 + k * gsz:o0 + 2 * k * gsz, 0:W], AL.add)
                    k >>= 1

            # ---- layer 1 ----
            l1_gat_ctx = tc.tile_pool(name="gat1", bufs=2)
            gat = l1_gat_ctx.__enter__()
            l2f_pool_ctx = tc.tile_pool(name="l2f", bufs=2)
            l2fp = l2f_pool_ctx.__enter__()
            GMAX = max(b1 - b0 + 1 for b0, b1, _ in groups)
            for i in range(2):
                lf = l2fp.tile([128, GMAX, L2_ROW], F16, tag="l2f")
                nc.vector.memset(lf[:, :, OUT_DIM + 1:OUT_DIM + 2], 1.0)

            for gi, (b0, b1, caps) in enumerate(groups):
                gsz = b1 - b0 + 1
                S_g = sum(caps) * gsz
                gt = gat.tile([128, S_g, L1_ROW], F16, tag="g")
                fire_gathers(gt, gi, caps, gsz, t1A, t1B, t1E, t1O, L1_ROW)

                # z = asrc + adst  (per bin, slot-major block-broadcast)
                z = sml.tile([128, S_g, HEADS], F16, tag="z")
                adb = ad_own[:, HEADS * b0:HEADS * (b1 + 1)].rearrange(
                    "p (g h) -> p g h", g=gsz)
                sofs = offs[gi][0]
                for x in range(4):
                    if caps[x] == 0:
                        continue
                    o0 = offs[gi][x] - sofs
                    dn = caps[x] * gsz
                    nc.vector.tensor_tensor(
                        z[:, o0:o0 + dn, :].rearrange(
                            "p (d g) h -> p d g h", g=gsz),
                        gt[:, o0:o0 + dn, L1H:L1H + HEADS].rearrange(
                            "p (d g) h -> p d g h", g=gsz),
                        adb.unsqueeze(1).broadcast_to(
                            [128, caps[x], gsz, HEADS]), AL.add)
                z2 = sml.tile([128, S_g, HEADS], F16, tag="z2")
                nc.vector.scalar_tensor_tensor(
                    z2[:].rearrange("p a b -> p (a b)"),
                    z[:].rearrange("p a b -> p (a b)"), NEG_SLOPE,
                    z[:].rearrange("p a b -> p (a b)"),
                    op0=AL.mult, op1=AL.max)
                wb = sml.tile([128, S_g, HEADS], F16, tag="wb")
                nc.scalar.activation(
                    wb[:].rearrange("p a b -> p (a b)"),
                    z2[:].rearrange("p a b -> p (a b)"), ACT.Exp)

                # in-place weight multiply over the whole 136-wide row:
                # k = 0..31 -> h (c-major interleave), k = 32 junk asrc,
                # k = 33 ones -> denominator.  Unit-stride fp16 everywhere.
                W = L1H + 2 * HEADS    # 136
                nc.vector.tensor_tensor(
                    gt[:, :, 0:W].rearrange("p s (k h) -> p s k h", h=HEADS),
                    gt[:, :, 0:W].rearrange("p s (k h) -> p s k h", h=HEADS),
                    wb[:].unsqueeze(2).broadcast_to(
                        [128, S_g, W // HEADS, HEADS]), AL.mult)

                # tree fold per bin (contiguous), then cross-bin sum
                parts = []
                for x in range(4):
                    if caps[x] == 0:
                        continue
                    o0 = offs[gi][x] - sofs
                    tree_fold(gt, o0, caps[x], gsz, W)
                    parts.append(gt[:, o0:o0 + gsz, 0:W])
                r = sml.tile([128, GMAX, W], F16, tag="r")
                if len(parts) == 1:
                    nc.vector.tensor_copy(r[:, 0:gsz, :], parts[0])
                else:
                    nc.vector.tensor_tensor(
                        r[:, 0:gsz, :], parts[0], parts[1], AL.add)
                for p_ in parts[2:]:
                    nc.vector.tensor_tensor(
                        r[:, 0:gsz, :], r[:, 0:gsz, :], p_, AL.add)

                rec = sml.tile([128, GMAX, HEADS], F32, tag="rec")
                nc.vector.reciprocal(
                    rec[:, 0:gsz, :],
                    r[:, 0:gsz, L1H + HEADS:L1H + 2 * HEADS])
                o1 = sml.tile([128, GMAX, L1H], F32, tag="o1")
                nc.vector.tensor_tensor(
                    o1[:, 0:gsz, :].rearrange("p g (c h) -> p g c h", h=HEADS),
                    r[:, 0:gsz, 0:L1H].rearrange("p g (c h) -> p g c h",
                                                 h=HEADS),
                    rec[:, 0:gsz, :].unsqueeze(2).broadcast_to(
                        [128, gsz, HID, HEADS]), AL.mult)
                if has_b1:
                    nc.vector.tensor_tensor(
                        o1[:, 0:gsz, :], o1[:, 0:gsz, :],
                        b1_t[:].unsqueeze(1).broadcast_to([128, gsz, L1H]),
                        AL.add)
                # elu(x) = max(x, exp(min(x, 0)) - 1); min via relu(-x) on ACT
                e1n = sml.tile([128, GMAX, L1H], F32, tag="e1n")
                nc.scalar.activation(
                    e1n[:, 0:gsz, :], o1[:, 0:gsz, :], ACT.Relu, scale=-1.0)
                e2 = sml.tile([128, GMAX, L1H], F32, tag="e2")
                nc.scalar.activation(
                    e2[:, 0:gsz, :], e1n[:, 0:gsz, :], ACT.Exp, scale=-1.0)
                elu = sml.tile([128, GMAX, L1H], F32, tag="elu")
                nc.vector.scalar_tensor_tensor(
                    elu[:, 0:gsz, :], e2[:, 0:gsz, :], -1.0,
                    o1[:, 0:gsz, :], op0=AL.add, op1=AL.max)

                # layer-2 fat rows: h2' = elu^T @ W2ext per block
                l2fat = l2fp.tile([128, GMAX, L2_ROW], F16, tag="l2f")
                for k in range(gsz):
                    b = b0 + k
                    tp = psp.tile([128, 128], F32, tag="tp")
                    nc.tensor.transpose(tp[:], elu[:, k, :], id_t[:])
                    eluT = sml.tile([128, 128], F16, tag="eluT")
                    nc.scalar.activation(eluT[:], tp[:], ACT.Copy)
                    h2p = psp.tile([128, W2N], F32, tag="h2p")
                    nc.tensor.matmul(h2p[:], eluT[:], w2e_t[:],
                                     start=True, stop=True)
                    nc.scalar.activation(
                        l2fat[:, k, 0:OUT_DIM + 1], h2p[:, 0:OUT_DIM + 1],
                        ACT.Copy)
                    nc.vector.tensor_copy(
                        ad2_own[:, b:b + 1], h2p[:, OUT_DIM + 1:OUT_DIM + 2])
                nrows = min(128 * gsz, NPC - 128 * b0)
                nfull = nrows // 128
                if nfull > 0:
                    nc.sync.dma_start(
                        cc2.ap()[128 * b0:128 * (b0 + nfull), :].rearrange(
                            "(t p) e -> p t e", p=128), l2fat[:, 0:nfull, :])
                rem = nrows - nfull * 128
                if rem > 0:
                    nc.sync.dma_start(
                        cc2.ap()[128 * (b0 + nfull):128 * (b0 + nfull) + rem,
                                 :], l2fat[0:rem, nfull, :])

            l2f_pool_ctx.__exit__(None, None, None)
            l1_gat_ctx.__exit__(None, None, None)
            tc.strict_bb_all_engine_barrier()
            nc.gpsimd.collective_compute(
                "AllGather", AL.bypass,
                replica_groups=[list(range(CORES))],
                ins=[cc2.ap().opt()], outs=[tbl2.ap().opt()])
            tc.strict_bb_all_engine_barrier()

            # ---- layer 2 ----
            t2A = tbl2.ap()[0:32768, :]
            t2B = tbl2.ap()[BBASE:TBL_ROWS, :]
            t2P = tbl2.ap().rearrange("(r two) e -> r (two e)", two=2)
            t2E = t2P[:, 0:L2_ROW]
            t2O = t2P[:, L2_ROW:2 * L2_ROW]
            W2R = OUT_DIM + 2      # reduce width: [wh2 | junk a2s | wsum]

            l2_gat_ctx = tc.tile_pool(name="gat2", bufs=2)
            gat = l2_gat_ctx.__enter__()
            for gi, (b0, b1, caps) in enumerate(groups):
                gsz = b1 - b0 + 1
                S_g = sum(caps) * gsz
                g2 = gat.tile([128, S_g, L2_ROW], F16, tag="g2")
                fire_gathers(g2, gi, caps, gsz, t2A, t2B, t2E, t2O, L2_ROW)

                z = sml.tile([128, S_g], F16, tag="z2l")
                ad2b = ad2_own[:, b0:b1 + 1]
                sofs = offs[gi][0]
                for x in range(4):
                    if caps[x] == 0:
                        continue
                    o0 = offs[gi][x] - sofs
                    nc.vector.tensor_tensor(
                        z[:, o0:o0 + caps[x] * gsz].rearrange(
                            "p (d g) -> p d g", g=gsz),
                        g2[:, o0:o0 + caps[x] * gsz, OUT_DIM].rearrange(
                            "p (d g) -> p d g", g=gsz),
                        ad2b.unsqueeze(1).broadcast_to([128, caps[x], gsz]),
                        AL.add)
                z2 = sml.tile([128, S_g], F16, tag="z2l2")
                nc.vector.scalar_tensor_tensor(
                    z2[:, :], z[:, :], NEG_SLOPE, z[:, :],
                    op0=AL.mult, op1=AL.max)
                w2t = sml.tile([128, S_g], F16, tag="w2t")
                nc.scalar.activation(w2t[:, :], z2[:, :], ACT.Exp)
                # duplicated weight pair for unit-stride broadcast multiply
                wp = sml.tile([128, S_g, 2], F16, tag="wp")
                nc.vector.tensor_copy(
                    wp[:], w2t[:].unsqueeze(2).broadcast_to([128, S_g, 2]))

                # single in-place multiply over cols [0:34]:
                # [h2(32) | junk a2s | one] as (k, 2) pairs
                nc.vector.tensor_tensor(
                    g2[:, :, 0:W2R].rearrange("p s (k t) -> p s k t", t=2),
                    g2[:, :, 0:W2R].rearrange("p s (k t) -> p s k t", t=2),
                    wp[:].unsqueeze(2).broadcast_to(
                        [128, S_g, W2R // 2, 2]), AL.mult)

                parts = []
                for x in range(4):
                    if caps[x] == 0:
                        continue
                    o0 = offs[gi][x] - sofs
                    tree_fold(g2, o0, caps[x], gsz, W2R)
                    parts.append(g2[:, o0:o0 + gsz, 0:W2R])
                r = sml.tile([128, GMAX, W2R], F16, tag="r2")
                if len(parts) == 1:
                    nc.vector.tensor_copy(r[:, 0:gsz, :], parts[0])
                else:
                    nc.vector.tensor_tensor(
                        r[:, 0:gsz, :], parts[0], parts[1], AL.add)
                for p_ in parts[2:]:
                    nc.vector.tensor_tensor(
                        r[:, 0:gsz, :], r[:, 0:gsz, :], p_, AL.add)

                rec = sml.tile([128, GMAX], F32, tag="rec2")
                nc.vector.reciprocal(
                    rec[:, 0:gsz], r[:, 0:gsz, OUT_DIM + 1])
                o2 = sml.tile([128, GMAX, OUT_DIM], F32, tag="o2")
                nc.vector.tensor_tensor(
                    o2[:, 0:gsz, :], r[:, 0:gsz, 0:OUT_DIM],
                    rec[:, 0:gsz].unsqueeze(2).broadcast_to(
                        [128, gsz, OUT_DIM]), AL.mult)
                nrows = min(128 * gsz, NPC - 128 * b0)
                nfull = nrows // 128
                if nfull > 0:
                    nc.sync.dma_start(
                        out.ap()[128 * b0:128 * (b0 + nfull), :].rearrange(
                            "(t p) e -> p t e", p=128), o2[:, 0:nfull, :])
                rem = nrows - nfull * 128
                if rem > 0:
                    nc.sync.dma_start(
                        out.ap()[128 * (b0 + nfull):128 * (b0 + nfull) + rem,
                                 :], o2[0:rem, nfull, :])

            l2_gat_ctx.__exit__(None, None, None)

    nc.compile()
    return nc


# ---------------------------------------------------------------------------
# weight prep + end-to-end run
# ---------------------------------------------------------------------------
def _run(x, edge_index, W1, a1_src, a1_dst, b1, W2, a2_src, a2_dst, b2,
         trace=False, n_nodes=None, bpc=None):
    x = np.asarray(x, dtype=np.float32)
    edge_index = np.asarray(edge_index)

    g = _prep_graph(edge_index, N)

    has_b1 = bool(np.abs(np.asarray(b1)).max() > 0)
    key = (5, has_b1, tuple((b0, b1, tuple(c)) for b0, b1, c in g["groups"]))
    if key in _CACHE:
        nc = _CACHE[key]
    else:
        nc = _build_program(g["groups"], g["offs"], g["tot_slots"], has_b1)
        _CACHE[key] = nc

    W1 = np.asarray(W1, np.float32)
    W2 = np.asarray(W2, np.float32)
    w1s = np.stack([W1[:, h * HID:(h + 1) * HID]
                    @ np.asarray(a1_src, np.float32)[h]
                    for h in range(HEADS)], axis=1)
    w1d = np.stack([W1[:, h * HID:(h + 1) * HID]
                    @ np.asarray(a1_dst, np.float32)[h]
                    for h in range(HEADS)], axis=1)
    W1i = W1.reshape(IN_DIM, HEADS, HID).transpose(0, 2, 1).reshape(
        IN_DIM, L1H)              # h cols in (c, h) interleave
    w1e_np = np.concatenate([W1i, w1s, w1d], axis=1)
    w2s = (W2 @ np.asarray(a2_src, np.float32)[0])[:, None]
    w2d = (W2 @ np.asarray(a2_dst, np.float32)[0])[:, None]
    w2e_np = np.concatenate([W2, w2s, w2d], axis=1)
    # rows of w2e follow the (c, h) interleave of layer-1 features
    w2e_np = w2e_np.reshape(HEADS, HID, W2N).transpose(1, 0, 2).reshape(
        L1H, W2N)

    xT = np.zeros((IN_DIM, TBL_ROWS), dtype=np.float32)
    xT[:, g["pos"]] = x.T

    common = {
        "w1e": w1e_np.astype(np.float16),
        "w2e": w2e_np.astype(np.float16),
        "b1t": np.tile(np.asarray(b1, np.float32).reshape(
            HEADS, HID).T.reshape(1, L1H), (128, 1)),
        "ident": np.eye(128, dtype=np.float32),
    }
    in_maps = []
    for c in range(CORES):
        in_maps.append({
            **common,
            "xTs": xT[:, c * STRIDE:(c + 1) * STRIDE].astype(np.float16),
            "idxt": g["idx"][c],
        })

    res = run_bass_kernel_spmd(nc, in_maps, list(range(CORES)), trace=trace)

    out_full = np.empty((N, OUT_DIM), dtype=np.float32)
    for c in range(CORES):
        out_full[g["nodes_of_core"][c]] = res.results[c]["out"][0:NPC]
    out_full += np.asarray(b2, np.float32)[None, :]
    return out_full, res


def kernel(x, edge_index, W1, a1_src, a1_dst, b1, W2, a2_src, a2_dst, b2):
    out, _ = _run(x, edge_index, W1, a1_src, a1_dst, b1, W2, a2_src, a2_dst,
                  b2)
    return out
